# revision 2
# baseline (speedup 1.0000x reference)
"""GCC-PHAT spatial likelihood grid kernel for Trainium2 (8 NeuronCores).

Self-contained: kernel(**inputs) -> np.ndarray. Shards the batch over the 8
cores (pure data parallel), runs a Bass/Tile kernel per core, gathers.
"""

import json

import numpy as np
from contextlib import ExitStack

import concourse.bass as bass
import concourse.bacc as bacc
import concourse.mybir as mybir
from concourse import tile, library_config
from concourse.bass_utils import run_bass_kernel_spmd
from concourse.tile import TileContext
from bass_rust import ScopedClock

# ---------------------------------------------------------------------------
# Workaround 1: this walrus build allows at most one semaphore wait per
# instruction. Post-process the BIR JSON: excess waits move onto NoOps
# inserted just before the offending instruction (same engine, so ordering
# is preserved).
_uid = [0]


def _fix_module(m):
    for f in m.get("functions", []):
        for bb in f.get("blocks", []):
            insts = bb.get("instructions")
            if not insts:
                continue
            out = []
            changed = False
            for ins in insts:
                si = ins.get("sync_info")
                ow = (si or {}).get("on_wait") or []
                if len(ow) > 1:
                    changed = True
                    for w in ow[1:]:
                        _uid[0] += 1
                        out.append({
                            "engine": ins["engine"], "ins": [], "outs": [],
                            "name": f"WFix-{_uid[0]}", "opcode": "NoOp",
                            "sync_info": {"on_update": [], "on_wait": [w]},
                        })
                    si["on_wait"] = ow[:1]
                out.append(ins)
            if changed:
                bb["instructions"] = out
    return m


_orig_to_json_bytes = bass.Bass.to_json_bytes


def _to_json_bytes(self):
    return json.dumps(_fix_module(json.loads(_orig_to_json_bytes(self)))).encode()


bass.Bass.to_json_bytes = _to_json_bytes

# Workaround 2: the TileContext tail Drain gets zero wait slots here; move
# the end-of-kernel waits onto SP NoOps emitted before the drain.


def _drain_and_barrier(self, tick_clock, wait_clock):
    nc = self.nc
    first_nop = nc.sync.nop()
    wait_clock.add_sem_waits(first_nop.ins, ScopedClock({None: tick_clock.global_clock}))
    si = first_nop.ins.sync_info
    if si is not None and len(si.on_wait) > 1:
        waits = list(si.on_wait)
        first_nop.ins.sync_info = mybir.SyncInfo(
            on_wait=waits[:1], on_update=list(si.on_update)
        )
        for w in waits[1:]:
            nop = nc.sync.nop()
            nop.ins.sync_info = mybir.SyncInfo(on_wait=[w], on_update=[])
    nc.sync.drain()
    nc.all_engine_barrier()
    popped = nc._tile_sem_poison_stack.pop()
    assert popped is self._sem_poison
    nc.clear_and_free_semaphores(list(self.sems.allocated().values()))
    nc.all_engine_barrier()


TileContext._drain_and_barrier = _drain_and_barrier

# ---------------------------------------------------------------------------
FP = mybir.dt.float32
I32 = mybir.dt.int32
I16 = mybir.dt.int16
ALU = mybir.AluOpType
ACTF = mybir.ActivationFunctionType

SR = 16000.0
SPEED = 343.0
G = 128
B, K, M, N = 16, 4, 4, 8192
P1, P2 = 128, 64  # N = P1*P2 ; n = p*64 + j ; bin k = k2*128 + k1
PAIRS = [(0, 1), (0, 2), (0, 3), (1, 2), (1, 3), (2, 3)]
NPA = 6
NB = 2
NBK = NB * K  # 8
NS = NBK * M  # 32
NPAIR = NBK * NPA  # 48
T1LO, T1HI = 56, 72
NT1 = T1HI - T1LO  # 16
NTAB = NT1 * P2  # 1024
NQ = G * G  # 16384
G2HI = G // 16  # 8


def make_consts():
    c = {}
    k1 = np.arange(P1)
    j = np.arange(P2)
    k2 = np.arange(P2)
    t2 = np.arange(P2)
    p = np.arange(P1)

    ang = 2 * np.pi * np.outer(p, k1) / P1
    c["a_re"] = np.cos(ang).astype(np.float32)  # [p, k1]
    c["a_im"] = (-np.sin(ang)).astype(np.float32)

    angt = 2 * np.pi * np.outer(k1, j) / N  # fwd twiddle e^{-i...}
    c["twf_re"] = np.tile(np.cos(angt).astype(np.float32), (1, NS))  # [128, 2048]
    c["twf_im"] = np.tile((-np.sin(angt)).astype(np.float32), (1, NS))

    angb = 2 * np.pi * np.outer(j, k2) / P2  # W64 = e^{-i...}
    c["b_re"] = np.cos(angb).astype(np.float32)  # [j, k2]
    c["b_im"] = (-np.sin(angb)).astype(np.float32)
    c["b_im_neg"] = -c["b_im"]

    kk = k1[:, None] + P1 * k2[None, :]  # [k1, k2] bin index
    o = np.arange(-5, 5)
    D = np.exp(2j * np.pi * kk[..., None] * o / N).sum(-1)
    DW = D * ((-1.0) ** kk) / N
    c["dw_re"] = np.tile(np.ascontiguousarray(DW.real).astype(np.float32), (1, NS))
    c["dw_im"] = np.tile(np.ascontiguousarray(DW.imag).astype(np.float32), (1, NS))

    angi = 2 * np.pi * np.outer(k2, t2) / P2  # E64 = e^{+i...}
    c["e64_re"] = np.cos(angi).astype(np.float32)  # [k2, t2]
    c["e64_im"] = np.sin(angi).astype(np.float32)
    c["e64_im_neg"] = -c["e64_im"]

    angti = 2 * np.pi * np.outer(k1, t2) / N  # inv twiddle e^{+i...}
    c["twi_re"] = np.tile(np.cos(angti).astype(np.float32), (1, NPAIR))  # [128, 3072]
    c["twi_im"] = np.tile(np.sin(angti).astype(np.float32), (1, NPAIR))

    t1 = np.arange(T1LO, T1HI)
    ango = 2 * np.pi * np.outer(k1, t1) / P1  # E128 = e^{+i...}
    c["e128_re"] = np.cos(ango).astype(np.float32)  # [k1, 16]
    c["e128_im_neg"] = (-np.sin(ango)).astype(np.float32)

    c["ident"] = np.eye(128, dtype=np.float32)

    # replication selectors: lhsT [48, 128]; set i: out row (16*g + r) <- tab row (g*6 + i)
    for i in range(NPA):
        r = np.zeros((NPAIR, 128), np.float32)
        for g in range(NBK):
            r[g * NPA + i, g * 16 : (g + 1) * 16] = 1.0
        c[f"repl{i}"] = r

    # grid parameter fields (wrapped layout): partition = (bk, g2lo), free = (g1, g2hi)
    t_lin = np.linspace(0.0, 1.0, G).astype(np.float32)
    g2lo = (p % 16)[:, None, None]
    g1 = np.arange(G)[None, :, None]
    g2hi = np.arange(G2HI)[None, None, :]
    gxc = np.broadcast_to(t_lin[g1], (128, G, G2HI))
    gyc = t_lin[(g2hi * 16 + g2lo)] * np.ones((128, G, G2HI), np.float32)
    c["gxc"] = np.ascontiguousarray(gxc.reshape(128, NQ // 16), np.float32)
    c["gyc"] = np.ascontiguousarray(gyc.reshape(128, NQ // 16), np.float32)
    return c


CONST_SPECS = [
    ("a_re", [128, 128]), ("a_im", [128, 128]),
    ("twf_re", [128, 64 * NS]), ("twf_im", [128, 64 * NS]),
    ("b_re", [64, 64]), ("b_im", [64, 64]), ("b_im_neg", [64, 64]),
    ("dw_re", [128, 64 * NS]), ("dw_im", [128, 64 * NS]),
    ("e64_re", [64, 64]), ("e64_im", [64, 64]), ("e64_im_neg", [64, 64]),
    ("twi_re", [128, 64 * NPAIR]), ("twi_im", [128, 64 * NPAIR]),
    ("e128_re", [128, NT1]), ("e128_im_neg", [128, NT1]),
    ("ident", [128, 128]),
    ("repl0", [NPAIR, 128]), ("repl1", [NPAIR, 128]), ("repl2", [NPAIR, 128]),
    ("repl3", [NPAIR, 128]), ("repl4", [NPAIR, 128]), ("repl5", [NPAIR, 128]),
    ("gxc", [128, NQ // 16]), ("gyc", [128, NQ // 16]),
]


def shard_core(signal, mic, room, core):
    b0 = core * NB
    sig = np.ascontiguousarray(signal[b0 : b0 + NB].reshape(NS, N))
    micc = mic[b0 : b0 + NB]
    roomc = room[b0 : b0 + NB]
    pp = np.arange(128)
    bk = pp // 16
    b = bk // K
    k = bk % K
    aux = np.zeros((128, 12), np.float32)
    aux[:, 0] = roomc[b, 0]
    aux[:, 1] = roomc[b, 1]
    for m in range(M):
        aux[:, 2 + m] = micc[b, k, m, 0]
        aux[:, 6 + m] = micc[b, k, m, 1]
    return {"signal": sig, "aux": aux}


def build_kernel():
    nc = bacc.Bacc("TRN2", target_bir_lowering=False, debug=False)
    sig_d = nc.declare_dram_parameter("signal", [NS, N], FP, isOutput=False)
    aux_d = nc.declare_dram_parameter("aux", [128, 12], FP, isOutput=False)
    cd = {
        name: nc.declare_dram_parameter(name, shape, FP, isOutput=False)
        for name, shape in CONST_SPECS
    }
    out_d = nc.declare_dram_parameter("out", [NBK, NQ], FP, isOutput=True)

    with tile.TileContext(nc) as tc:
        with ExitStack() as top:
            # small constants that live for the whole kernel
            cpool = top.enter_context(tc.tile_pool(name="consts", bufs=1))
            SMALL = {"a_re", "a_im", "b_re", "b_im", "b_im_neg", "e64_re", "e64_im",
                     "e64_im_neg", "e128_re", "e128_im_neg", "ident",
                     "repl0", "repl1", "repl2", "repl3", "repl4", "repl5"}
            ct = {}
            for name, shape in CONST_SPECS:
                if name in SMALL:
                    t = cpool.tile(shape, FP, tag=name, name=name)
                    nc.sync.dma_start(t[:], cd[name][:, :])
                    ct[name] = t
            aux = cpool.tile([128, 12], FP, tag="aux", name="aux")
            nc.sync.dma_start(aux[:], aux_d[:, :])
            bias_round = cpool.tile([128, 1], FP, tag="bias_round", name="bias_round")
            nc.vector.memset(bias_round[:], 512.0)
            bias_eps = cpool.tile([128, 1], FP, tag="bias_eps", name="bias_eps")
            nc.vector.memset(bias_eps[:], 1e-18)

            # gather inputs persist until the gather phase
            gpool = top.enter_context(tc.tile_pool(name="gin", bufs=1))
            gdata = [gpool.tile([128, NTAB], FP, tag=f"gdata{i}", name=f"gdata{i}") for i in range(NPA)]
            gidx = [gpool.tile([128, NTAB], I16, tag=f"gidx{i}", name=f"gidx{i}") for i in range(NPA)]

            # ---------------- geometry / index pipeline ----------------
            with ExitStack() as geo:
                gp = geo.enter_context(tc.tile_pool(name="geo", bufs=1))
                NF = NQ // 16  # 1024
                gxc = gp.tile([128, NF], FP, tag="gxc", name="gxc")
                nc.sync.dma_start(gxc[:], cd["gxc"][:, :])
                gyc = gp.tile([128, NF], FP, tag="gyc", name="gyc")
                nc.sync.dma_start(gyc[:], cd["gyc"][:, :])
                gx = gp.tile([128, NF], FP, tag="gx", name="gx")
                gy = gp.tile([128, NF], FP, tag="gy", name="gy")
                nc.vector.tensor_scalar_mul(gx[:], gxc[:], aux[:, 0:1])
                nc.vector.tensor_scalar_mul(gy[:], gyc[:], aux[:, 1:2])
                dist = [gp.tile([128, NF], FP, tag=f"dist{m}", name=f"dist{m}") for m in range(M)]
                negm = gp.tile([128, 8], FP, tag="negm", name="negm")
                nc.vector.tensor_scalar_mul(negm[:], aux[:, 2:10], -1.0)
                for m in range(M):
                    d2 = gp.tile([128, NF], FP, tag="d2", name="d2")
                    dy2 = gp.tile([128, NF], FP, tag="dy2", name="dy2")
                    nc.scalar.activation(d2[:], gx[:], ACTF.Square, bias=negm[:, m : m + 1])
                    nc.scalar.activation(
                        dy2[:], gy[:], ACTF.Square, bias=negm[:, 4 + m : 5 + m]
                    )
                    nc.vector.tensor_add(d2[:], d2[:], dy2[:])
                    nc.scalar.activation(dist[m][:], d2[:], ACTF.Sqrt)
                for i, (mi, mj) in enumerate(PAIRS):
                    ds = gp.tile([128, NF], FP, tag="ds", name="ds")
                    dsi = gp.tile([128, NF], I32, tag="dsi", name="dsi")
                    d16 = gp.tile([128, NF], I32, tag="d16", name="d16")
                    dl = gp.tile([128, NF], I32, tag="dl", name="dl")
                    nc.vector.tensor_sub(ds[:], dist[mi][:], dist[mj][:])
                    # dlocal+0.5 = ds*(SR/343) + 512.5 ; then clamp, truncate
                    nc.scalar.activation(
                        ds[:], ds[:], ACTF.Identity,
                        bias=bias_round[:, 0:1], scale=float(np.float32(SR) / np.float32(SPEED)),
                    )
                    nc.vector.tensor_scalar(
                        ds[:], ds[:], 0.0, 1023.0, op0=ALU.max, op1=ALU.min
                    )
                    nc.vector.tensor_copy(dsi[:], ds[:])
                    # permuted index d' = ((d*16) & 1023) + ((d*16) >> 10)
                    nc.vector.tensor_scalar(d16[:], dsi[:], 16, None, op0=ALU.mult)
                    nc.vector.tensor_scalar(dl[:], d16[:], 1023, None, op0=ALU.bitwise_and)
                    nc.vector.tensor_scalar(
                        d16[:], d16[:], 10, None, op0=ALU.logical_shift_right
                    )
                    nc.vector.tensor_tensor(gidx[i][:], d16[:], dl[:], op=ALU.add)

            # ---------------- forward FFT + spectra + inverse ----------------
            class MPool:
                def __init__(self, name, space="SBUF", bufs=1, side=None):
                    self.cm = tc.tile_pool(name=name, bufs=bufs, space=space, side=side)
                    self.pool = self.cm.__enter__()
                def tile(self, *a, **kw):
                    return self.pool.tile(*a, **kw)
                def close(self):
                    self.cm.__exit__(None, None, None)

            # step A + twiddle
            p_ytw = MPool("p_ytw", side="left")
            ytw_re = p_ytw.tile([128, NS * P2], FP, name="ytw_re")
            ytw_im = p_ytw.tile([128, NS * P2], FP, name="ytw_im")
            p_a = MPool("p_a", side="left")
            ps_a = MPool("ps_a", space="PSUM", bufs=2)
            x2 = p_a.tile([128, NS, P2], FP, name="x2")
            nc.sync.dma_start(x2[:], sig_d[:, :].rearrange("s (p j) -> p s j", p=128))
            twf_re = p_a.tile([128, P2], FP, name="twf_re")
            nc.sync.dma_start(twf_re[:], cd["twf_re"][:, 0:P2])
            twf_im = p_a.tile([128, P2], FP, name="twf_im")
            nc.sync.dma_start(twf_im[:], cd["twf_im"][:, 0:P2])
            x2f = x2[:].rearrange("p s j -> p (s j)")
            twfr_b = twf_re[:].rearrange("p (o j) -> p o j", o=1).broadcast_to((128, 8, P2))
            twfi_b = twf_im[:].rearrange("p (o j) -> p o j", o=1).broadcast_to((128, 8, P2))
            ytw_re3 = ytw_re[:].rearrange("p (s j) -> p s j", j=P2)
            ytw_im3 = ytw_im[:].rearrange("p (s j) -> p s j", j=P2)
            for chunk in range(4):
                sl = slice(chunk * 512, (chunk + 1) * 512)
                ssl = slice(chunk * 8, (chunk + 1) * 8)
                yre = ps_a.tile([128, 512], FP, tag="yre", name="yre")
                yim = ps_a.tile([128, 512], FP, tag="yim", name="yim")
                nc.tensor.matmul(yre[:], ct["a_re"][:], x2f[:, sl])
                nc.tensor.matmul(yim[:], ct["a_im"][:], x2f[:, sl])
                tmp = p_a.tile([128, 512], FP, tag="twtmp", name="twtmp")
                yre3 = yre[:].rearrange("p (s j) -> p s j", j=P2)
                yim3 = yim[:].rearrange("p (s j) -> p s j", j=P2)
                tmp3 = tmp[:].rearrange("p (s j) -> p s j", j=P2)
                nc.vector.tensor_mul(tmp3, yre3, twfr_b)
                nc.vector.tensor_mul(ytw_re3[:, ssl], yim3, twfi_b)
                nc.vector.tensor_sub(ytw_re3[:, ssl], tmp3, ytw_re3[:, ssl])
                nc.vector.tensor_mul(tmp3, yre3, twfi_b)
                nc.vector.tensor_mul(ytw_im3[:, ssl], yim3, twfr_b)
                nc.vector.tensor_add(ytw_im3[:, ssl], tmp3, ytw_im3[:, ssl])
            ps_a.close()
            p_a.close()

            # transposes -> ytT
            p_yt = MPool("p_yt", side="right")
            ps_t = MPool("ps_t", space="PSUM", bufs=4)
            ytT_re = p_yt.tile([64, NS * 128], FP, name="ytT_re")
            ytT_im = p_yt.tile([64, NS * 128], FP, name="ytT_im")
            for s4 in range(NS // 4):
                for src_, dst in [(ytw_re, ytT_re), (ytw_im, ytT_im)]:
                    pt = ps_t.tile([64, 512], FP, tag="ptr", name="ptr")
                    for k in range(4):
                        s = s4 * 4 + k
                        nc.tensor.transpose(
                            pt[:, k * 128 : (k + 1) * 128],
                            src_[:, s * 64 : (s + 1) * 64], ct["ident"][:],
                        )
                    nc.scalar.copy(dst[:, s4 * 512 : (s4 + 1) * 512], pt[:])
            ps_t.close()
            p_ytw.close()

            # step B -> Z
            p_z = MPool("p_z", side="left")
            ps_b = MPool("ps_b", space="PSUM", bufs=2)
            z_re = p_z.tile([128, NS * P2], FP, name="z_re")
            z_im = p_z.tile([128, NS * P2], FP, name="z_im")
            for sc in range(4):
                zre = ps_b.tile([128, 512], FP, tag="zre", name="zre")
                zim = ps_b.tile([128, 512], FP, tag="zim", name="zim")
                for si in range(8):
                    s = sc * 8 + si
                    lre = ytT_re[:, s * 128 : (s + 1) * 128]
                    lim = ytT_im[:, s * 128 : (s + 1) * 128]
                    osl = slice(si * 64, (si + 1) * 64)
                    nc.tensor.matmul(zre[:, osl], lre, ct["b_re"][:], start=True, stop=False)
                    nc.tensor.matmul(zre[:, osl], lim, ct["b_im_neg"][:], start=False, stop=True)
                    nc.tensor.matmul(zim[:, osl], lre, ct["b_im"][:], start=True, stop=False)
                    nc.tensor.matmul(zim[:, osl], lim, ct["b_re"][:], start=False, stop=True)
                sl = slice(sc * 512, (sc + 1) * 512)
                nc.scalar.copy(z_re[:, sl], zre[:])
                nc.scalar.copy(z_im[:, sl], zim[:])
            ps_b.close()
            p_yt.close()

            # PHAT per mic + Dirichlet -> U, V
            p_uv = MPool("p_uv", side="right")
            u_re = p_uv.tile([128, NS * P2], FP, name="u_re")
            u_im = p_uv.tile([128, NS * P2], FP, name="u_im")
            v_re = p_uv.tile([128, NS * P2], FP, name="v_re")
            v_im = p_uv.tile([128, NS * P2], FP, name="v_im")
            p_n = MPool("p_n", side="right")
            dw_res = p_n.tile([128, P2], FP, name="dw_res")
            nc.sync.dma_start(dw_res[:], cd["dw_re"][:, 0:P2])
            dw_ims = p_n.tile([128, P2], FP, name="dw_ims")
            nc.sync.dma_start(dw_ims[:], cd["dw_im"][:, 0:P2])
            dw_re3 = dw_res[:].rearrange("p (o j) -> p o j", o=1).broadcast_to((128, NS, P2))
            dw_im3 = dw_ims[:].rearrange("p (o j) -> p o j", o=1).broadcast_to((128, NS, P2))
            nrm = p_n.tile([128, NS * P2], FP, name="nrm")
            tmp2 = p_n.tile([128, NS * P2], FP, name="tmp2")
            nc.vector.tensor_mul(nrm[:], z_re[:], z_re[:])
            nc.vector.tensor_mul(tmp2[:], z_im[:], z_im[:])
            nc.vector.tensor_add(nrm[:], nrm[:], tmp2[:])
            nc.scalar.activation(nrm[:], nrm[:], ACTF.Sqrt, bias=bias_eps[:, 0:1])
            nc.vector.reciprocal(nrm[:], nrm[:])
            nc.vector.tensor_mul(u_re[:], z_re[:], nrm[:])
            nc.vector.tensor_mul(u_im[:], z_im[:], nrm[:])
            ur3 = u_re[:].rearrange("p (s j) -> p s j", j=P2)
            ui3 = u_im[:].rearrange("p (s j) -> p s j", j=P2)
            vr3 = v_re[:].rearrange("p (s j) -> p s j", j=P2)
            vi3 = v_im[:].rearrange("p (s j) -> p s j", j=P2)
            tm3 = tmp2[:].rearrange("p (s j) -> p s j", j=P2)
            nc.vector.tensor_mul(vr3, ur3, dw_re3)
            nc.vector.tensor_mul(tm3, ui3, dw_im3)
            nc.vector.tensor_sub(vr3, vr3, tm3)
            nc.vector.tensor_mul(vi3, ur3, dw_im3)
            nc.vector.tensor_mul(tm3, ui3, dw_re3)
            nc.vector.tensor_add(vi3, vi3, tm3)
            p_n.close()
            p_z.close()

            # pair cross spectra Q = V_i * conj(U_j)
            p_q = MPool("p_q", side="left")
            q_re = p_q.tile([128, NBK, NPA, P2], FP, name="q_re")
            q_im = p_q.tile([128, NBK, NPA, P2], FP, name="q_im")
            qtmp = p_q.tile([128, NBK, 3, P2], FP, name="qtmp")
            u4_re = u_re[:].rearrange("p (bk m k) -> p bk m k", bk=NBK, m=M)
            u4_im = u_im[:].rearrange("p (bk m k) -> p bk m k", bk=NBK, m=M)
            v4_re = v_re[:].rearrange("p (bk m k) -> p bk m k", bk=NBK, m=M)
            v4_im = v_im[:].rearrange("p (bk m k) -> p bk m k", bk=NBK, m=M)
            for gi, js, qoff in [(0, [1, 2, 3], 0), (1, [2, 3], 3), (2, [3], 5)]:
                ng = len(js)
                j0 = js[0]
                sh = (128, NBK, ng, P2)
                vi_re = v4_re[:, :, gi : gi + 1, :].broadcast_to(sh)
                vi_im = v4_im[:, :, gi : gi + 1, :].broadcast_to(sh)
                uj_re = u4_re[:, :, j0 : j0 + ng, :]
                uj_im = u4_im[:, :, j0 : j0 + ng, :]
                oq_re = q_re[:, :, qoff : qoff + ng, :]
                oq_im = q_im[:, :, qoff : qoff + ng, :]
                tq = qtmp[:, :, 0:ng, :]
                nc.vector.tensor_mul(oq_re, vi_re, uj_re)
                nc.vector.tensor_mul(tq, vi_im, uj_im)
                nc.vector.tensor_add(oq_re, oq_re, tq)
                nc.vector.tensor_mul(oq_im, vi_im, uj_re)
                nc.vector.tensor_mul(tq, vi_re, uj_im)
                nc.vector.tensor_sub(oq_im, oq_im, tq)
            p_uv.close()

            # transpose Q -> Qt [k2=64, (pair, k1)]
            p_qt = MPool("p_qt", side="right")
            ps_q = MPool("ps_q", space="PSUM", bufs=4)
            qt_re = p_qt.tile([64, NPAIR * 128], FP, name="qt_re")
            qt_im = p_qt.tile([64, NPAIR * 128], FP, name="qt_im")
            qf_re = q_re[:].rearrange("p bk pr k -> p (bk pr k)")
            qf_im = q_im[:].rearrange("p bk pr k -> p (bk pr k)")
            for p4 in range(NPAIR // 4):
                for src_, dst in [(qf_re, qt_re), (qf_im, qt_im)]:
                    pt = ps_q.tile([64, 512], FP, tag="ptq", name="ptq")
                    for k in range(4):
                        pr = p4 * 4 + k
                        nc.tensor.transpose(
                            pt[:, k * 128 : (k + 1) * 128],
                            src_[:, pr * 64 : (pr + 1) * 64], ct["ident"][:],
                        )
                    nc.scalar.copy(dst[:, p4 * 512 : (p4 + 1) * 512], pt[:])
            ps_q.close()
            p_q.close()

            # inverse inner + twiddle -> TwInner
            p_in = MPool("p_in", side="left")
            p_tw = MPool("p_tw", side="left")
            ps_i = MPool("ps_i", space="PSUM", bufs=2)
            in_re = p_in.tile([128, NPAIR * P2], FP, name="in_re")
            in_im = p_in.tile([128, NPAIR * P2], FP, name="in_im")
            twi_res = p_tw.tile([128, P2], FP, name="twi_res")
            nc.sync.dma_start(twi_res[:], cd["twi_re"][:, 0:P2])
            twi_ims = p_tw.tile([128, P2], FP, name="twi_ims")
            nc.sync.dma_start(twi_ims[:], cd["twi_im"][:, 0:P2])
            twir_b = twi_res[:].rearrange("p (o t) -> p o t", o=1).broadcast_to((128, 8, P2))
            twii_b = twi_ims[:].rearrange("p (o t) -> p o t", o=1).broadcast_to((128, 8, P2))
            for pc in range(NPAIR // 8):
                ire = ps_i.tile([128, 512], FP, tag="ire", name="ire")
                iim = ps_i.tile([128, 512], FP, tag="iim", name="iim")
                for pi in range(8):
                    pr = pc * 8 + pi
                    lre = qt_re[:, pr * 128 : (pr + 1) * 128]
                    lim = qt_im[:, pr * 128 : (pr + 1) * 128]
                    osl = slice(pi * 64, (pi + 1) * 64)
                    nc.tensor.matmul(ire[:, osl], lre, ct["e64_re"][:], start=True, stop=False)
                    nc.tensor.matmul(ire[:, osl], lim, ct["e64_im_neg"][:], start=False, stop=True)
                    nc.tensor.matmul(iim[:, osl], lre, ct["e64_im"][:], start=True, stop=False)
                    nc.tensor.matmul(iim[:, osl], lim, ct["e64_re"][:], start=False, stop=True)
                sl = slice(pc * 512, (pc + 1) * 512)
                t_a = p_tw.tile([128, 512], FP, tag="t_a", name="t_a")
                ire3 = ire[:].rearrange("p (r t) -> p r t", t=P2)
                iim3 = iim[:].rearrange("p (r t) -> p r t", t=P2)
                ta3 = t_a[:].rearrange("p (r t) -> p r t", t=P2)
                inre3 = in_re[:, sl].rearrange("p (r t) -> p r t", t=P2)
                inim3 = in_im[:, sl].rearrange("p (r t) -> p r t", t=P2)
                nc.vector.tensor_mul(ta3, ire3, twir_b)
                nc.vector.tensor_mul(inre3, iim3, twii_b)
                nc.vector.tensor_sub(inre3, ta3, inre3)
                nc.vector.tensor_mul(ta3, ire3, twii_b)
                nc.vector.tensor_mul(inim3, iim3, twir_b)
                nc.vector.tensor_add(inim3, inim3, ta3)
            ps_i.close()
            p_tw.close()
            p_qt.close()

            # inverse outer -> Tt ; tables ; replication
            p_to = MPool("p_to", side="right")
            ps_o = MPool("ps_o", space="PSUM", bufs=1)
            tt = p_to.tile([NT1, NPAIR * P2], FP, name="tt")
            for oc in range(NPAIR * P2 // 512):
                ot = ps_o.tile([NT1, 512], FP, tag="ot", name="ot")
                sl = slice(oc * 512, (oc + 1) * 512)
                nc.tensor.matmul(ot[:], ct["e128_re"][:], in_re[:, sl], start=True, stop=False)
                nc.tensor.matmul(ot[:], ct["e128_im_neg"][:], in_im[:, sl], start=False, stop=True)
                nc.scalar.copy(tt[:, sl], ot[:])
            tabs = p_to.tile([NPAIR, NTAB], FP, name="tabs")
            tt3 = tt[:].rearrange("a (pr t) -> a pr t", pr=NPAIR)
            ptab = ps_o.tile([NPAIR, NTAB], FP, name="ptab")
            for t2v in range(P2):
                nc.tensor.transpose(
                    ptab[:, t2v * NT1 : (t2v + 1) * NT1],
                    tt3[:, :, t2v : t2v + 1],
                    ct["ident"][0:16, 0:16],
                )
            nc.scalar.copy(tabs[:], ptab[:])
            for i in range(NPA):
                prep = ps_o.tile([128, NTAB], FP, tag="prep", name="prep")
                for half in range(2):
                    sl = slice(half * 512, (half + 1) * 512)
                    nc.tensor.matmul(prep[:, sl], ct[f"repl{i}"][:], tabs[:, sl])
                nc.scalar.copy(gdata[i][:], prep[:])
            ps_o.close()
            p_in.close()
            p_to.close()

            # ---------------- gathers + accumulate + normalize + out -------
            with ExitStack() as gph:
                op = gph.enter_context(tc.tile_pool(name="gout", bufs=1))
                acc = op.tile([128, NQ], FP, tag="acc", name="acc")
                for i in range(NPA):
                    gout = op.tile([128, NQ], FP, tag="gout", name="gout")
                    with tc.tile_critical():
                        if i == 0:
                            nc.gpsimd.load_library(library_config.ap_gather)
                        nc.gpsimd.ap_gather(
                            gout[:], gdata[i][:], gidx[i][:],
                            channels=128, num_elems=NTAB, d=1, num_idxs=NQ,
                        )
                    if i == 0:
                        nc.vector.tensor_copy(acc[:], gout[:])
                    else:
                        nc.vector.tensor_add(acc[:], acc[:], gout[:])

                accv = acc[:].rearrange("(g r) q -> g r q", r=16)[:, 0, :]  # [8, NQ]
                accc = op.tile([NBK, NQ], FP, tag="gout", name="accc")
                nc.sync.dma_start(accc[:], accv)
                mx = op.tile([NBK, 1], FP, tag="mx", name="mx")
                nc.vector.tensor_reduce(
                    mx[:], accc[:], axis=mybir.AxisListType.X, op=ALU.max
                )
                nc.vector.reciprocal(mx[:], mx[:])
                grids = op.tile([NBK, 2048], FP, tag="grids", name="grids")
                for ch in range(NQ // 2048):
                    sl = slice(ch * 2048, (ch + 1) * 2048)
                    nc.scalar.activation(
                        grids[:], accc[:, sl], ACTF.Identity, scale=mx[:, 0:1]
                    )
                    nc.sync.dma_start(out_d[:, sl], grids[:])

    nc.compile()
    return nc


_NC_CACHE = {}


def kernel(signal, mic_coordinates, room_dims):
    signal = np.ascontiguousarray(np.asarray(signal, dtype=np.float32))
    mic_coordinates = np.ascontiguousarray(np.asarray(mic_coordinates, dtype=np.float32))
    room_dims = np.ascontiguousarray(np.asarray(room_dims, dtype=np.float32))
    if "nc" not in _NC_CACHE:
        _NC_CACHE["nc"] = build_kernel()
        _NC_CACHE["consts"] = make_consts()
    nc = _NC_CACHE["nc"]
    consts = _NC_CACHE["consts"]
    in_maps = []
    for core in range(8):
        m = shard_core(signal, mic_coordinates, room_dims, core)
        m.update(consts)
        in_maps.append(m)
    res = run_bass_kernel_spmd(nc, in_maps, core_ids=list(range(8)), trace=False)
    outs = [res.results[c]["out"].reshape(NB * K, NQ) for c in range(8)]
    return np.concatenate(outs, axis=0).reshape(B, K, NQ).astype(np.float32)



# revision 3
# speedup vs baseline: 1.0189x; 1.0189x over previous
"""GCC-PHAT spatial likelihood grid kernel for Trainium2 (8 NeuronCores).

Self-contained: kernel(**inputs) -> np.ndarray. Shards the batch over the 8
cores (pure data parallel), runs a Bass/Tile kernel per core, gathers.
"""

import json

import numpy as np
from contextlib import ExitStack

import concourse.bass as bass
import concourse.bacc as bacc
import concourse.mybir as mybir
from concourse import tile, library_config
from concourse.bass_utils import run_bass_kernel_spmd
from concourse.tile import TileContext
from bass_rust import ScopedClock

# ---------------------------------------------------------------------------
# Workaround 1: this walrus build allows at most one semaphore wait per
# instruction. Post-process the BIR JSON: excess waits move onto NoOps
# inserted just before the offending instruction (same engine, so ordering
# is preserved).
_uid = [0]


def _fix_module(m):
    for f in m.get("functions", []):
        for bb in f.get("blocks", []):
            insts = bb.get("instructions")
            if not insts:
                continue
            out = []
            changed = False
            for ins in insts:
                si = ins.get("sync_info")
                ow = (si or {}).get("on_wait") or []
                if len(ow) > 1:
                    changed = True
                    for w in ow[1:]:
                        _uid[0] += 1
                        out.append({
                            "engine": ins["engine"], "ins": [], "outs": [],
                            "name": f"WFix-{_uid[0]}", "opcode": "NoOp",
                            "sync_info": {"on_update": [], "on_wait": [w]},
                        })
                    si["on_wait"] = ow[:1]
                out.append(ins)
            if changed:
                bb["instructions"] = out
    return m


_orig_to_json_bytes = bass.Bass.to_json_bytes


def _to_json_bytes(self):
    return json.dumps(_fix_module(json.loads(_orig_to_json_bytes(self)))).encode()


bass.Bass.to_json_bytes = _to_json_bytes

# Workaround 2: the TileContext tail Drain gets zero wait slots here; move
# the end-of-kernel waits onto SP NoOps emitted before the drain.


def _drain_and_barrier(self, tick_clock, wait_clock):
    nc = self.nc
    first_nop = nc.sync.nop()
    wait_clock.add_sem_waits(first_nop.ins, ScopedClock({None: tick_clock.global_clock}))
    si = first_nop.ins.sync_info
    if si is not None and len(si.on_wait) > 1:
        waits = list(si.on_wait)
        first_nop.ins.sync_info = mybir.SyncInfo(
            on_wait=waits[:1], on_update=list(si.on_update)
        )
        for w in waits[1:]:
            nop = nc.sync.nop()
            nop.ins.sync_info = mybir.SyncInfo(on_wait=[w], on_update=[])
    nc.sync.drain()
    nc.all_engine_barrier()
    popped = nc._tile_sem_poison_stack.pop()
    assert popped is self._sem_poison
    nc.clear_and_free_semaphores(list(self.sems.allocated().values()))
    nc.all_engine_barrier()


TileContext._drain_and_barrier = _drain_and_barrier

# ---------------------------------------------------------------------------
FP = mybir.dt.float32
I32 = mybir.dt.int32
I16 = mybir.dt.int16
ALU = mybir.AluOpType
ACTF = mybir.ActivationFunctionType

SR = 16000.0
SPEED = 343.0
G = 128
B, K, M, N = 16, 4, 4, 8192
P1, P2 = 128, 64  # N = P1*P2 ; n = p*64 + j ; bin k = k2*128 + k1
PAIRS = [(0, 1), (0, 2), (0, 3), (1, 2), (1, 3), (2, 3)]
NPA = 6
NB = 2
NBK = NB * K  # 8
NS = NBK * M  # 32
NPAIR = NBK * NPA  # 48
T1LO, T1HI = 56, 72
NT1 = T1HI - T1LO  # 16
NTAB = NT1 * P2  # 1024
NQ = G * G  # 16384
G2HI = G // 16  # 8


def make_consts():
    c = {}
    k1 = np.arange(P1)
    j = np.arange(P2)
    k2 = np.arange(P2)
    t2 = np.arange(P2)
    p = np.arange(P1)

    ang = 2 * np.pi * np.outer(p, k1) / P1
    c["a_re"] = np.cos(ang).astype(np.float32)  # [p, k1]
    c["a_im"] = (-np.sin(ang)).astype(np.float32)

    angt = 2 * np.pi * np.outer(k1, j) / N  # fwd twiddle e^{-i...}
    c["twf_re"] = np.tile(np.cos(angt).astype(np.float32), (1, NS))  # [128, 2048]
    c["twf_im"] = np.tile((-np.sin(angt)).astype(np.float32), (1, NS))

    angb = 2 * np.pi * np.outer(j, k2) / P2  # W64 = e^{-i...}
    c["b_re"] = np.cos(angb).astype(np.float32)  # [j, k2]
    c["b_im"] = (-np.sin(angb)).astype(np.float32)
    c["b_im_neg"] = -c["b_im"]

    kk = k1[:, None] + P1 * k2[None, :]  # [k1, k2] bin index
    o = np.arange(-5, 5)
    D = np.exp(2j * np.pi * kk[..., None] * o / N).sum(-1)
    DW = D * ((-1.0) ** kk) / N
    c["dw_re"] = np.tile(np.ascontiguousarray(DW.real).astype(np.float32), (1, NS))
    c["dw_im"] = np.tile(np.ascontiguousarray(DW.imag).astype(np.float32), (1, NS))

    angi = 2 * np.pi * np.outer(k2, t2) / P2  # E64 = e^{+i...}
    c["e64_re"] = np.cos(angi).astype(np.float32)  # [k2, t2]
    c["e64_im"] = np.sin(angi).astype(np.float32)
    c["e64_im_neg"] = -c["e64_im"]

    angti = 2 * np.pi * np.outer(k1, t2) / N  # inv twiddle e^{+i...}
    c["twi_re"] = np.tile(np.cos(angti).astype(np.float32), (1, NPAIR))  # [128, 3072]
    c["twi_im"] = np.tile(np.sin(angti).astype(np.float32), (1, NPAIR))

    t1 = np.arange(T1LO, T1HI)
    ango = 2 * np.pi * np.outer(k1, t1) / P1  # E128 = e^{+i...}
    c["e128_re"] = np.cos(ango).astype(np.float32)  # [k1, 16]
    c["e128_im_neg"] = (-np.sin(ango)).astype(np.float32)

    c["ident"] = np.eye(128, dtype=np.float32)

    # group-broadcast selector: lhsT [8, 128]; out row p <- in row p//16
    r16 = np.zeros((NBK, 128), np.float32)
    for g in range(NBK):
        r16[g, g * 16 : (g + 1) * 16] = 1.0
    c["repl16"] = r16

    # grid parameter fields (wrapped layout): partition = (bk, g2lo), free = (g1, g2hi)
    t_lin = np.linspace(0.0, 1.0, G).astype(np.float32)
    g2lo = (p % 16)[:, None, None]
    g1 = np.arange(G)[None, :, None]
    g2hi = np.arange(G2HI)[None, None, :]
    gxc = np.broadcast_to(t_lin[g1], (128, G, G2HI))
    gyc = t_lin[(g2hi * 16 + g2lo)] * np.ones((128, G, G2HI), np.float32)
    c["gxc"] = np.ascontiguousarray(gxc.reshape(128, NQ // 16), np.float32)
    c["gyc"] = np.ascontiguousarray(gyc.reshape(128, NQ // 16), np.float32)
    return c


CONST_SPECS = [
    ("a_re", [128, 128]), ("a_im", [128, 128]),
    ("twf_re", [128, 64 * NS]), ("twf_im", [128, 64 * NS]),
    ("b_re", [64, 64]), ("b_im", [64, 64]), ("b_im_neg", [64, 64]),
    ("dw_re", [128, 64 * NS]), ("dw_im", [128, 64 * NS]),
    ("e64_re", [64, 64]), ("e64_im", [64, 64]), ("e64_im_neg", [64, 64]),
    ("twi_re", [128, 64 * NPAIR]), ("twi_im", [128, 64 * NPAIR]),
    ("e128_re", [128, NT1]), ("e128_im_neg", [128, NT1]),
    ("ident", [128, 128]),
    ("repl16", [NBK, 128]),
    ("gxc", [128, NQ // 16]), ("gyc", [128, NQ // 16]),
]


def shard_core(signal, mic, room, core):
    b0 = core * NB
    sig = np.ascontiguousarray(signal[b0 : b0 + NB].transpose(2, 0, 1, 3).reshape(NS, N))
    micc = mic[b0 : b0 + NB]
    roomc = room[b0 : b0 + NB]
    pp = np.arange(128)
    bk = pp // 16
    b = bk // K
    k = bk % K
    aux = np.zeros((128, 12), np.float32)
    aux[:, 0] = roomc[b, 0]
    aux[:, 1] = roomc[b, 1]
    for m in range(M):
        aux[:, 2 + m] = micc[b, k, m, 0]
        aux[:, 6 + m] = micc[b, k, m, 1]
    return {"signal": sig, "aux": aux}


def build_kernel():
    nc = bacc.Bacc("TRN2", target_bir_lowering=False, debug=False)
    sig_d = nc.declare_dram_parameter("signal", [NS, N], FP, isOutput=False)
    aux_d = nc.declare_dram_parameter("aux", [128, 12], FP, isOutput=False)
    cd = {
        name: nc.declare_dram_parameter(name, shape, FP, isOutput=False)
        for name, shape in CONST_SPECS
    }
    out_d = nc.declare_dram_parameter("out", [NBK, NQ], FP, isOutput=True)

    with tile.TileContext(nc) as tc:
        with ExitStack() as top:
            # small constants that live for the whole kernel
            cpool = top.enter_context(tc.tile_pool(name="consts", bufs=1))
            SMALL = {"a_re", "a_im", "b_re", "b_im", "b_im_neg", "e64_re", "e64_im",
                     "e64_im_neg", "e128_re", "e128_im_neg", "ident", "repl16"}
            ct = {}
            for name, shape in CONST_SPECS:
                if name in SMALL:
                    t = cpool.tile(shape, FP, tag=name, name=name)
                    nc.sync.dma_start(t[:], cd[name][:, :])
                    ct[name] = t
            aux = cpool.tile([128, 12], FP, tag="aux", name="aux")
            nc.sync.dma_start(aux[:], aux_d[:, :])
            bias_round = cpool.tile([128, 1], FP, tag="bias_round", name="bias_round")
            nc.vector.memset(bias_round[:], 512.0)
            bias_eps = cpool.tile([128, 1], FP, tag="bias_eps", name="bias_eps")
            nc.vector.memset(bias_eps[:], 1e-18)

            # gather inputs persist until the gather phase
            gpool = top.enter_context(tc.tile_pool(name="gin", bufs=1))
            gdata = [gpool.tile([128, NTAB], FP, tag=f"gdata{i}", name=f"gdata{i}") for i in range(NPA)]
            gidx = [gpool.tile([128, NTAB], I16, tag=f"gidx{i}", name=f"gidx{i}") for i in range(NPA)]

            # ---------------- geometry / index pipeline ----------------
            with ExitStack() as geo:
                gp = geo.enter_context(tc.tile_pool(name="geo", bufs=1))
                NF = NQ // 16  # 1024
                gxc = gp.tile([128, NF], FP, tag="gxc", name="gxc")
                nc.sync.dma_start(gxc[:], cd["gxc"][:, :])
                gyc = gp.tile([128, NF], FP, tag="gyc", name="gyc")
                nc.sync.dma_start(gyc[:], cd["gyc"][:, :])
                gx = gp.tile([128, NF], FP, tag="gx", name="gx")
                gy = gp.tile([128, NF], FP, tag="gy", name="gy")
                nc.vector.tensor_scalar_mul(gx[:], gxc[:], aux[:, 0:1])
                nc.vector.tensor_scalar_mul(gy[:], gyc[:], aux[:, 1:2])
                dist = [gp.tile([128, NF], FP, tag=f"dist{m}", name=f"dist{m}") for m in range(M)]
                negm = gp.tile([128, 8], FP, tag="negm", name="negm")
                nc.vector.tensor_scalar_mul(negm[:], aux[:, 2:10], -1.0)
                for m in range(M):
                    d2 = gp.tile([128, NF], FP, tag="d2", name="d2")
                    dy2 = gp.tile([128, NF], FP, tag="dy2", name="dy2")
                    nc.scalar.activation(d2[:], gx[:], ACTF.Square, bias=negm[:, m : m + 1])
                    nc.scalar.activation(
                        dy2[:], gy[:], ACTF.Square, bias=negm[:, 4 + m : 5 + m]
                    )
                    nc.vector.tensor_add(d2[:], d2[:], dy2[:])
                    nc.scalar.activation(dist[m][:], d2[:], ACTF.Sqrt)
                for i, (mi, mj) in enumerate(PAIRS):
                    ds = gp.tile([128, NF], FP, tag="ds", name="ds")
                    dsi = gp.tile([128, NF], I32, tag="dsi", name="dsi")
                    d16 = gp.tile([128, NF], I32, tag="d16", name="d16")
                    dl = gp.tile([128, NF], I32, tag="dl", name="dl")
                    nc.vector.tensor_sub(ds[:], dist[mi][:], dist[mj][:])
                    # dlocal+0.5 = ds*(SR/343) + 512.5 ; then clamp, truncate
                    nc.scalar.activation(
                        ds[:], ds[:], ACTF.Identity,
                        bias=bias_round[:, 0:1], scale=float(np.float32(SR) / np.float32(SPEED)),
                    )
                    nc.vector.tensor_scalar(
                        ds[:], ds[:], 0.0, 1023.0, op0=ALU.max, op1=ALU.min
                    )
                    nc.vector.tensor_copy(dsi[:], ds[:])
                    # permuted index d' = ((d*16) & 1023) + ((d*16) >> 10)
                    nc.vector.tensor_scalar(d16[:], dsi[:], 16, None, op0=ALU.mult)
                    nc.vector.tensor_scalar(dl[:], d16[:], 1023, None, op0=ALU.bitwise_and)
                    nc.vector.tensor_scalar(
                        d16[:], d16[:], 10, None, op0=ALU.logical_shift_right
                    )
                    nc.vector.tensor_tensor(gidx[i][:], d16[:], dl[:], op=ALU.add)

            # ------------- forward FFT + spectra + inverse (half/pair split) ----
            # Signal order is now s = m*NBK + bk (mics major), so half h of the
            # signal range covers mics {2h, 2h+1}. Front tiles are split per
            # half and the post-PHAT stages per pair so that pair 0's table
            # (and its gather) is ready long before the full front finishes.
            class MPool:
                def __init__(self, name, space="SBUF", bufs=1, side=None):
                    self.cm = tc.tile_pool(name=name, bufs=bufs, space=space, side=side)
                    self.pool = self.cm.__enter__()
                def tile(self, *a, **kw):
                    return self.pool.tile(*a, **kw)
                def close(self):
                    self.cm.__exit__(None, None, None)

            NSH = NS // 2  # 16 signals per half
            HC = NSH * P2  # 1024 cols per half

            # step A + twiddle
            p_ytw = MPool("p_ytw", side="left")
            ytw_re = [p_ytw.tile([128, HC], FP, name=f"ytw_re{h}") for h in range(2)]
            ytw_im = [p_ytw.tile([128, HC], FP, name=f"ytw_im{h}") for h in range(2)]
            p_a = MPool("p_a", side="left")
            ps_a = MPool("ps_a", space="PSUM", bufs=2)
            x2 = p_a.tile([128, NS, P2], FP, name="x2")
            nc.sync.dma_start(x2[:], sig_d[:, :].rearrange("s (p j) -> p s j", p=128))
            twf_re = p_a.tile([128, P2], FP, name="twf_re")
            nc.sync.dma_start(twf_re[:], cd["twf_re"][:, 0:P2])
            twf_im = p_a.tile([128, P2], FP, name="twf_im")
            nc.sync.dma_start(twf_im[:], cd["twf_im"][:, 0:P2])
            x2f = x2[:].rearrange("p s j -> p (s j)")
            twfr_b = twf_re[:].rearrange("p (o j) -> p o j", o=1).broadcast_to((128, 8, P2))
            twfi_b = twf_im[:].rearrange("p (o j) -> p o j", o=1).broadcast_to((128, 8, P2))
            for chunk in range(4):
                h, lc = chunk // 2, chunk % 2
                sl = slice(chunk * 512, (chunk + 1) * 512)
                ssl = slice(lc * 8, lc * 8 + 8)
                yre = ps_a.tile([128, 512], FP, tag="yre", name="yre")
                yim = ps_a.tile([128, 512], FP, tag="yim", name="yim")
                nc.tensor.matmul(yre[:], ct["a_re"][:], x2f[:, sl])
                nc.tensor.matmul(yim[:], ct["a_im"][:], x2f[:, sl])
                tmp = p_a.tile([128, 512], FP, tag="twtmp", name="twtmp")
                yre3 = yre[:].rearrange("p (s j) -> p s j", j=P2)
                yim3 = yim[:].rearrange("p (s j) -> p s j", j=P2)
                tmp3 = tmp[:].rearrange("p (s j) -> p s j", j=P2)
                ytw_re3 = ytw_re[h][:].rearrange("p (s j) -> p s j", j=P2)
                ytw_im3 = ytw_im[h][:].rearrange("p (s j) -> p s j", j=P2)
                nc.vector.tensor_mul(tmp3, yre3, twfr_b)
                nc.vector.tensor_mul(ytw_re3[:, ssl], yim3, twfi_b)
                nc.vector.tensor_sub(ytw_re3[:, ssl], tmp3, ytw_re3[:, ssl])
                nc.vector.tensor_mul(tmp3, yre3, twfi_b)
                nc.vector.tensor_mul(ytw_im3[:, ssl], yim3, twfr_b)
                nc.vector.tensor_add(ytw_im3[:, ssl], tmp3, ytw_im3[:, ssl])
            ps_a.close()
            p_a.close()

            # transposes -> ytT (per half)
            p_yt = MPool("p_yt", side="right")
            ps_t = MPool("ps_t", space="PSUM", bufs=4)
            ytT_re = [p_yt.tile([64, NSH * 128], FP, name=f"ytT_re{h}") for h in range(2)]
            ytT_im = [p_yt.tile([64, NSH * 128], FP, name=f"ytT_im{h}") for h in range(2)]
            for s4 in range(NS // 4):
                h = s4 // 4
                for srcs, dsts in [(ytw_re, ytT_re), (ytw_im, ytT_im)]:
                    pt = ps_t.tile([64, 512], FP, tag="ptr", name="ptr")
                    for k in range(4):
                        ls = (s4 % 4) * 4 + k
                        nc.tensor.transpose(
                            pt[:, k * 128 : (k + 1) * 128],
                            srcs[h][:, ls * 64 : (ls + 1) * 64], ct["ident"][:],
                        )
                    nc.scalar.copy(dsts[h][:, (s4 % 4) * 512 : (s4 % 4 + 1) * 512], pt[:])
            ps_t.close()
            p_ytw.close()

            # step B -> Z (per half)
            p_z = MPool("p_z", side="left")
            ps_b = MPool("ps_b", space="PSUM", bufs=2)
            z_re = [p_z.tile([128, HC], FP, name=f"z_re{h}") for h in range(2)]
            z_im = [p_z.tile([128, HC], FP, name=f"z_im{h}") for h in range(2)]
            for sc in range(4):
                h, lc = sc // 2, sc % 2
                zre = ps_b.tile([128, 512], FP, tag="zre", name="zre")
                zim = ps_b.tile([128, 512], FP, tag="zim", name="zim")
                for si in range(8):
                    ls = lc * 8 + si
                    lre = ytT_re[h][:, ls * 128 : (ls + 1) * 128]
                    lim = ytT_im[h][:, ls * 128 : (ls + 1) * 128]
                    osl = slice(si * 64, (si + 1) * 64)
                    nc.tensor.matmul(zre[:, osl], lre, ct["b_re"][:], start=True, stop=False)
                    nc.tensor.matmul(zre[:, osl], lim, ct["b_im_neg"][:], start=False, stop=True)
                    nc.tensor.matmul(zim[:, osl], lre, ct["b_im"][:], start=True, stop=False)
                    nc.tensor.matmul(zim[:, osl], lim, ct["b_re"][:], start=False, stop=True)
                sl = slice(lc * 512, (lc + 1) * 512)
                nc.scalar.copy(z_re[h][:, sl], zre[:])
                nc.scalar.copy(z_im[h][:, sl], zim[:])
            ps_b.close()
            p_yt.close()

            # PHAT per mic + Dirichlet -> U, V (per half)
            p_uv = MPool("p_uv", side="right")
            u_re = [p_uv.tile([128, HC], FP, name=f"u_re{h}") for h in range(2)]
            u_im = [p_uv.tile([128, HC], FP, name=f"u_im{h}") for h in range(2)]
            v_re = [p_uv.tile([128, HC], FP, name=f"v_re{h}") for h in range(2)]
            v_im = [p_uv.tile([128, HC], FP, name=f"v_im{h}") for h in range(2)]
            p_n = MPool("p_n", side="right", bufs=2)
            dw_res = p_n.tile([128, P2], FP, tag="dwr", name="dw_res")
            nc.sync.dma_start(dw_res[:], cd["dw_re"][:, 0:P2])
            dw_ims = p_n.tile([128, P2], FP, tag="dwi", name="dw_ims")
            nc.sync.dma_start(dw_ims[:], cd["dw_im"][:, 0:P2])
            dw_re3 = dw_res[:].rearrange("p (o j) -> p o j", o=1).broadcast_to((128, NSH, P2))
            dw_im3 = dw_ims[:].rearrange("p (o j) -> p o j", o=1).broadcast_to((128, NSH, P2))
            for h in range(2):
                nrm = p_n.tile([128, HC], FP, tag="nrm", name="nrm")
                tmp2 = p_n.tile([128, HC], FP, tag="tmp2", name="tmp2")
                nc.vector.tensor_mul(nrm[:], z_re[h][:], z_re[h][:])
                nc.vector.tensor_mul(tmp2[:], z_im[h][:], z_im[h][:])
                nc.vector.tensor_add(nrm[:], nrm[:], tmp2[:])
                nc.scalar.activation(nrm[:], nrm[:], ACTF.Sqrt, bias=bias_eps[:, 0:1])
                nc.vector.reciprocal(nrm[:], nrm[:])
                nc.vector.tensor_mul(u_re[h][:], z_re[h][:], nrm[:])
                nc.vector.tensor_mul(u_im[h][:], z_im[h][:], nrm[:])
                ur3 = u_re[h][:].rearrange("p (s j) -> p s j", j=P2)
                ui3 = u_im[h][:].rearrange("p (s j) -> p s j", j=P2)
                vr3 = v_re[h][:].rearrange("p (s j) -> p s j", j=P2)
                vi3 = v_im[h][:].rearrange("p (s j) -> p s j", j=P2)
                tm3 = tmp2[:].rearrange("p (s j) -> p s j", j=P2)
                nc.vector.tensor_mul(vr3, ur3, dw_re3)
                nc.vector.tensor_mul(tm3, ui3, dw_im3)
                nc.vector.tensor_sub(vr3, vr3, tm3)
                nc.vector.tensor_mul(vi3, ur3, dw_im3)
                nc.vector.tensor_mul(tm3, ui3, dw_re3)
                nc.vector.tensor_add(vi3, vi3, tm3)
            p_n.close()
            p_z.close()

            # pair cross spectra Q = V_i * conj(U_j), one tile per pair
            p_q = MPool("p_q", side="left", bufs=1)
            q_re = [p_q.tile([128, NBK, P2], FP, tag=f"q_re{i}", name=f"q_re{i}") for i in range(NPA)]
            q_im = [p_q.tile([128, NBK, P2], FP, tag=f"q_im{i}", name=f"q_im{i}") for i in range(NPA)]

            def mview(tiles, m):
                return tiles[m // 2][:].rearrange(
                    "p (m2 bk k) -> p m2 bk k", m2=2, bk=NBK
                )[:, m % 2]

            for i, (mi, mj) in enumerate(PAIRS):
                vi_re, vi_im = mview(v_re, mi), mview(v_im, mi)
                uj_re, uj_im = mview(u_re, mj), mview(u_im, mj)
                tq = p_q.tile([128, NBK, P2], FP, tag="qtmp", name="qtmp")
                nc.vector.tensor_mul(q_re[i][:], vi_re, uj_re)
                nc.vector.tensor_mul(tq[:], vi_im, uj_im)
                nc.vector.tensor_add(q_re[i][:], q_re[i][:], tq[:])
                nc.vector.tensor_mul(q_im[i][:], vi_im, uj_re)
                nc.vector.tensor_mul(tq[:], vi_re, uj_im)
                nc.vector.tensor_sub(q_im[i][:], q_im[i][:], tq[:])
            p_uv.close()

            # per-pair back half: transpose Q -> inverse inner+twiddle ->
            # outer -> per-pair table -> replicate -> gather (2 half-gathers)
            p_bk = MPool("p_bk", side="right", bufs=2)
            ps_k = MPool("ps_k", space="PSUM", bufs=2)
            ps_o = MPool("ps_o", space="PSUM", bufs=1)
            p_go = MPool("p_go", side="left", bufs=1)
            twi_res = p_bk.tile([128, P2], FP, tag="twir", name="twi_res")
            nc.sync.dma_start(twi_res[:], cd["twi_re"][:, 0:P2])
            twi_ims = p_bk.tile([128, P2], FP, tag="twii", name="twi_ims")
            nc.sync.dma_start(twi_ims[:], cd["twi_im"][:, 0:P2])
            twir_b = twi_res[:].rearrange("p (o t) -> p o t", o=1).broadcast_to((128, 8, P2))
            twii_b = twi_ims[:].rearrange("p (o t) -> p o t", o=1).broadcast_to((128, 8, P2))
            acc = p_go.tile([128, NQ], FP, tag="acc", name="acc")
            NQH = NQ // 2

            for i in range(NPA):
                # transpose Q_i -> Qt_i [64 (k2), NBK*128]
                qt_re = p_bk.tile([64, NBK * 128], FP, tag="qt_re", name=f"qt_re{i}")
                qt_im = p_bk.tile([64, NBK * 128], FP, tag="qt_im", name=f"qt_im{i}")
                qf_re = q_re[i][:].rearrange("p bk k -> p (bk k)")
                qf_im = q_im[i][:].rearrange("p bk k -> p (bk k)")
                for b4 in range(2):
                    for srcf, dst in [(qf_re, qt_re), (qf_im, qt_im)]:
                        pt = ps_k.tile([64, 512], FP, tag="ptq", name="ptq")
                        for k in range(4):
                            bkk = b4 * 4 + k
                            nc.tensor.transpose(
                                pt[:, k * 128 : (k + 1) * 128],
                                srcf[:, bkk * 64 : (bkk + 1) * 64], ct["ident"][:],
                            )
                        nc.scalar.copy(dst[:, b4 * 512 : (b4 + 1) * 512], pt[:])
                # inverse inner + twiddle -> TwInner_i [128, NBK*P2]
                ire = ps_o.tile([128, 512], FP, tag="ire", name="ire")
                iim = ps_o.tile([128, 512], FP, tag="iim", name="iim")
                for pi in range(NBK):
                    lre = qt_re[:, pi * 128 : (pi + 1) * 128]
                    lim = qt_im[:, pi * 128 : (pi + 1) * 128]
                    osl = slice(pi * 64, (pi + 1) * 64)
                    nc.tensor.matmul(ire[:, osl], lre, ct["e64_re"][:], start=True, stop=False)
                    nc.tensor.matmul(ire[:, osl], lim, ct["e64_im_neg"][:], start=False, stop=True)
                    nc.tensor.matmul(iim[:, osl], lre, ct["e64_im"][:], start=True, stop=False)
                    nc.tensor.matmul(iim[:, osl], lim, ct["e64_re"][:], start=False, stop=True)
                in_re = p_bk.tile([128, 512], FP, tag="in_re", name=f"in_re{i}")
                in_im = p_bk.tile([128, 512], FP, tag="in_im", name=f"in_im{i}")
                t_a = p_bk.tile([128, 512], FP, tag="t_a", name="t_a")
                ire3 = ire[:].rearrange("p (r t) -> p r t", t=P2)
                iim3 = iim[:].rearrange("p (r t) -> p r t", t=P2)
                ta3 = t_a[:].rearrange("p (r t) -> p r t", t=P2)
                inre3 = in_re[:].rearrange("p (r t) -> p r t", t=P2)
                inim3 = in_im[:].rearrange("p (r t) -> p r t", t=P2)
                nc.vector.tensor_mul(ta3, ire3, twir_b)
                nc.vector.tensor_mul(inre3, iim3, twii_b)
                nc.vector.tensor_sub(inre3, ta3, inre3)
                nc.vector.tensor_mul(ta3, ire3, twii_b)
                nc.vector.tensor_mul(inim3, iim3, twir_b)
                nc.vector.tensor_add(inim3, inim3, ta3)
                # inverse outer -> tt_i [NT1, NBK*P2]
                ot = ps_o.tile([NT1, 512], FP, tag="ot", name="ot")
                nc.tensor.matmul(ot[:], ct["e128_re"][:], in_re[:], start=True, stop=False)
                nc.tensor.matmul(ot[:], ct["e128_im_neg"][:], in_im[:], start=False, stop=True)
                tt_i = p_bk.tile([NT1, 512], FP, tag="tt", name=f"tt{i}")
                nc.scalar.copy(tt_i[:], ot[:])
                # per-pair table: [NBK, NTAB] via 64 small transposes
                tt3 = tt_i[:].rearrange("a (bk t) -> a bk t", bk=NBK)
                ptab = ps_o.tile([NBK, NTAB], FP, tag="ptab", name="ptab")
                for t2v in range(P2):
                    nc.tensor.transpose(
                        ptab[:, t2v * NT1 : (t2v + 1) * NT1],
                        tt3[:, :, t2v : t2v + 1],
                        ct["ident"][0:NT1, 0:NT1],
                    )
                tabs_i = p_bk.tile([NBK, NTAB], FP, tag="tabs", name=f"tabs{i}")
                nc.scalar.copy(tabs_i[:], ptab[:])
                # replicate each bk row across its 16 partitions
                for halfc in range(2):
                    sl = slice(halfc * 512, (halfc + 1) * 512)
                    prep = ps_o.tile([128, 512], FP, tag="prep", name="prep")
                    nc.tensor.matmul(prep[:], ct["repl16"][:], tabs_i[:, sl])
                    nc.scalar.copy(gdata[i][:, sl], prep[:])
                # gather in two halves (keeps the bounce buffer at 32KB)
                for gh in range(2):
                    hsl = slice(gh * NQH, (gh + 1) * NQH)
                    gout = p_go.tile([128, NQH], FP, tag="gout", name="gout")
                    if i == 0 and gh == 0:
                        nc.gpsimd.load_library(library_config.ap_gather)
                    nc.gpsimd.ap_gather(
                        gout[:], gdata[i][:], gidx[i][:, gh * 512 : (gh + 1) * 512],
                        channels=128, num_elems=NTAB, d=1, num_idxs=NQH,
                    )
                    if i == 0:
                        nc.vector.tensor_copy(acc[:, hsl], gout[:])
                    else:
                        nc.vector.tensor_add(acc[:, hsl], acc[:, hsl], gout[:])
            ps_o.close()
            ps_k.close()
            p_bk.close()

            # ---------------- normalize + out ----------------
            with ExitStack() as gph:
                op = gph.enter_context(tc.tile_pool(name="gout2", bufs=1))
                psn = gph.enter_context(tc.tile_pool(name="psn", bufs=1, space="PSUM"))
                m128 = op.tile([128, 1], FP, tag="m128", name="m128")
                nc.vector.tensor_reduce(
                    m128[:], acc[:], axis=mybir.AxisListType.X, op=ALU.max
                )
                mt = psn.tile([1, 128], FP, tag="mt", name="mt")
                nc.tensor.transpose(mt[:], m128[:], ct["ident"][:])
                mts = op.tile([1, 128], FP, tag="mts", name="mts")
                nc.scalar.copy(mts[:], mt[:])
                mg = op.tile([1, NBK], FP, tag="mg", name="mg")
                nc.vector.tensor_reduce(
                    mg[:],
                    mts[:].rearrange("a (g r) -> a g r", r=16),
                    axis=mybir.AxisListType.X, op=ALU.max,
                )
                nc.vector.reciprocal(mg[:], mg[:])
                mgt = psn.tile([NBK, 1], FP, tag="mgt", name="mgt")
                nc.tensor.transpose(mgt[:], mg[:], ct["ident"][0:1, 0:1])
                mgs = op.tile([NBK, 1], FP, tag="mgs", name="mgs")
                nc.scalar.copy(mgs[:], mgt[:])
                scp = psn.tile([128, 1], FP, tag="scp", name="scp")
                nc.tensor.matmul(scp[:], ct["repl16"][:], mgs[:])
                sc = op.tile([128, 1], FP, tag="sc", name="sc")
                nc.scalar.copy(sc[:], scp[:])
                for ch in range(NQ // 2048):
                    sl = slice(ch * 2048, (ch + 1) * 2048)
                    grids = op.tile([128, 2048], FP, tag="grids", name="grids")
                    nc.scalar.activation(
                        grids[:], acc[:, sl], ACTF.Copy, scale=sc[:, 0:1]
                    )
                    gv = grids[:].rearrange("(g r) q -> g r q", r=16)[:, 0, :]
                    nc.sync.dma_start(out_d[:, sl], gv)
            p_go.close()
            p_q.close()

    nc.compile()
    return nc


_NC_CACHE = {}


def kernel(signal, mic_coordinates, room_dims):
    signal = np.ascontiguousarray(np.asarray(signal, dtype=np.float32))
    mic_coordinates = np.ascontiguousarray(np.asarray(mic_coordinates, dtype=np.float32))
    room_dims = np.ascontiguousarray(np.asarray(room_dims, dtype=np.float32))
    if "nc" not in _NC_CACHE:
        _NC_CACHE["nc"] = build_kernel()
        _NC_CACHE["consts"] = make_consts()
    nc = _NC_CACHE["nc"]
    consts = _NC_CACHE["consts"]
    in_maps = []
    for core in range(8):
        m = shard_core(signal, mic_coordinates, room_dims, core)
        m.update(consts)
        in_maps.append(m)
    res = run_bass_kernel_spmd(nc, in_maps, core_ids=list(range(8)), trace=False)
    outs = [res.results[c]["out"].reshape(NB * K, NQ) for c in range(8)]
    return np.concatenate(outs, axis=0).reshape(B, K, NQ).astype(np.float32)



# revision 4
# speedup vs baseline: 1.0321x; 1.0130x over previous
"""GCC-PHAT spatial likelihood grid kernel for Trainium2 (8 NeuronCores).

Self-contained: kernel(**inputs) -> np.ndarray. Shards the batch over the 8
cores (pure data parallel), runs a Bass/Tile kernel per core, gathers.
"""

import json

import numpy as np
from contextlib import ExitStack

import concourse.bass as bass
import concourse.bacc as bacc
import concourse.mybir as mybir
from concourse import tile, library_config
from concourse.bass_utils import run_bass_kernel_spmd
from concourse.tile import TileContext
from bass_rust import ScopedClock

# ---------------------------------------------------------------------------
# Workaround 1: this walrus build allows at most one semaphore wait per
# instruction. Post-process the BIR JSON: excess waits move onto NoOps
# inserted just before the offending instruction (same engine, so ordering
# is preserved).
_uid = [0]


def _fix_module(m):
    for f in m.get("functions", []):
        for bb in f.get("blocks", []):
            insts = bb.get("instructions")
            if not insts:
                continue
            out = []
            changed = False
            for ins in insts:
                si = ins.get("sync_info")
                ow = (si or {}).get("on_wait") or []
                if len(ow) > 1:
                    changed = True
                    for w in ow[1:]:
                        _uid[0] += 1
                        out.append({
                            "engine": ins["engine"], "ins": [], "outs": [],
                            "name": f"WFix-{_uid[0]}", "opcode": "NoOp",
                            "sync_info": {"on_update": [], "on_wait": [w]},
                        })
                    si["on_wait"] = ow[:1]
                out.append(ins)
            if changed:
                bb["instructions"] = out
    return m


_orig_to_json_bytes = bass.Bass.to_json_bytes


def _to_json_bytes(self):
    return json.dumps(_fix_module(json.loads(_orig_to_json_bytes(self)))).encode()


bass.Bass.to_json_bytes = _to_json_bytes

# Workaround 2: the TileContext tail Drain gets zero wait slots here; move
# the end-of-kernel waits onto SP NoOps emitted before the drain.


def _drain_and_barrier(self, tick_clock, wait_clock):
    nc = self.nc
    first_nop = nc.sync.nop()
    wait_clock.add_sem_waits(first_nop.ins, ScopedClock({None: tick_clock.global_clock}))
    si = first_nop.ins.sync_info
    if si is not None and len(si.on_wait) > 1:
        waits = list(si.on_wait)
        first_nop.ins.sync_info = mybir.SyncInfo(
            on_wait=waits[:1], on_update=list(si.on_update)
        )
        for w in waits[1:]:
            nop = nc.sync.nop()
            nop.ins.sync_info = mybir.SyncInfo(on_wait=[w], on_update=[])
    nc.sync.drain()
    nc.all_engine_barrier()
    popped = nc._tile_sem_poison_stack.pop()
    assert popped is self._sem_poison
    nc.clear_and_free_semaphores(list(self.sems.allocated().values()))
    nc.all_engine_barrier()


TileContext._drain_and_barrier = _drain_and_barrier

# ---------------------------------------------------------------------------
FP = mybir.dt.float32
I32 = mybir.dt.int32
I16 = mybir.dt.int16
ALU = mybir.AluOpType
ACTF = mybir.ActivationFunctionType

SR = 16000.0
SPEED = 343.0
G = 128
B, K, M, N = 16, 4, 4, 8192
P1, P2 = 128, 64  # N = P1*P2 ; n = p*64 + j ; bin k = k2*128 + k1
PAIRS = [(0, 1), (0, 2), (0, 3), (1, 2), (1, 3), (2, 3)]
NPA = 6
NB = 2
NBK = NB * K  # 8
NS = NBK * M  # 32
NPAIR = NBK * NPA  # 48
T1LO, T1HI = 56, 72
NT1 = T1HI - T1LO  # 16
NTAB = NT1 * P2  # 1024
NQ = G * G  # 16384
G2HI = G // 16  # 8


def make_consts():
    c = {}
    k1 = np.arange(P1)
    j = np.arange(P2)
    k2 = np.arange(P2)
    t2 = np.arange(P2)
    p = np.arange(P1)

    ang = 2 * np.pi * np.outer(p, k1) / P1
    c["a_re"] = np.cos(ang).astype(np.float32)  # [p, k1]
    c["a_im"] = (-np.sin(ang)).astype(np.float32)

    angt = 2 * np.pi * np.outer(k1, j) / N  # fwd twiddle e^{-i...}
    c["twf_re"] = np.tile(np.cos(angt).astype(np.float32), (1, NS))  # [128, 2048]
    c["twf_im"] = np.tile((-np.sin(angt)).astype(np.float32), (1, NS))

    angb = 2 * np.pi * np.outer(j, k2) / P2  # W64 = e^{-i...}
    c["b_re"] = np.cos(angb).astype(np.float32)  # [j, k2]
    c["b_im"] = (-np.sin(angb)).astype(np.float32)
    c["b_im_neg"] = -c["b_im"]

    kk = k1[:, None] + P1 * k2[None, :]  # [k1, k2] bin index
    o = np.arange(-5, 5)
    D = np.exp(2j * np.pi * kk[..., None] * o / N).sum(-1)
    DW = D * ((-1.0) ** kk) / N
    c["dw_re"] = np.tile(np.ascontiguousarray(DW.real).astype(np.float32), (1, NS))
    c["dw_im"] = np.tile(np.ascontiguousarray(DW.imag).astype(np.float32), (1, NS))

    angi = 2 * np.pi * np.outer(k2, t2) / P2  # E64 = e^{+i...}
    c["e64_re"] = np.cos(angi).astype(np.float32)  # [k2, t2]
    c["e64_im"] = np.sin(angi).astype(np.float32)
    c["e64_im_neg"] = -c["e64_im"]

    angti = 2 * np.pi * np.outer(k1, t2) / N  # inv twiddle e^{+i...}
    c["twi_re"] = np.tile(np.cos(angti).astype(np.float32), (1, NPAIR))  # [128, 3072]
    c["twi_im"] = np.tile(np.sin(angti).astype(np.float32), (1, NPAIR))

    t1 = np.arange(T1LO, T1HI)
    ango = 2 * np.pi * np.outer(k1, t1) / P1  # E128 = e^{+i...}
    c["e128_re"] = np.cos(ango).astype(np.float32)  # [k1, 16]
    c["e128_im_neg"] = (-np.sin(ango)).astype(np.float32)

    c["ident"] = np.eye(128, dtype=np.float32)

    # group-broadcast selector: lhsT [8, 128]; out row p <- in row p//16
    r16 = np.zeros((NBK, 128), np.float32)
    for g in range(NBK):
        r16[g, g * 16 : (g + 1) * 16] = 1.0
    c["repl16"] = r16

    # grid parameter fields (wrapped layout): partition = (bk, g2lo), free = (g1, g2hi)
    t_lin = np.linspace(0.0, 1.0, G).astype(np.float32)
    g2lo = (p % 16)[:, None, None]
    g1 = np.arange(G)[None, :, None]
    g2hi = np.arange(G2HI)[None, None, :]
    gxc = np.broadcast_to(t_lin[g1], (128, G, G2HI))
    gyc = t_lin[(g2hi * 16 + g2lo)] * np.ones((128, G, G2HI), np.float32)
    c["gxc"] = np.ascontiguousarray(gxc.reshape(128, NQ // 16), np.float32)
    c["gyc"] = np.ascontiguousarray(gyc.reshape(128, NQ // 16), np.float32)
    return c


CONST_SPECS = [
    ("a_re", [128, 128]), ("a_im", [128, 128]),
    ("twf_re", [128, 64 * NS]), ("twf_im", [128, 64 * NS]),
    ("b_re", [64, 64]), ("b_im", [64, 64]), ("b_im_neg", [64, 64]),
    ("dw_re", [128, 64 * NS]), ("dw_im", [128, 64 * NS]),
    ("e64_re", [64, 64]), ("e64_im", [64, 64]), ("e64_im_neg", [64, 64]),
    ("twi_re", [128, 64 * NPAIR]), ("twi_im", [128, 64 * NPAIR]),
    ("e128_re", [128, NT1]), ("e128_im_neg", [128, NT1]),
    ("ident", [128, 128]),
    ("repl16", [NBK, 128]),
    ("gxc", [128, NQ // 16]), ("gyc", [128, NQ // 16]),
]


def shard_core(signal, mic, room, core):
    b0 = core * NB
    sig = np.ascontiguousarray(signal[b0 : b0 + NB].transpose(2, 0, 1, 3).reshape(NS, N))
    micc = mic[b0 : b0 + NB]
    roomc = room[b0 : b0 + NB]
    pp = np.arange(128)
    bk = pp // 16
    b = bk // K
    k = bk % K
    aux = np.zeros((128, 12), np.float32)
    aux[:, 0] = roomc[b, 0]
    aux[:, 1] = roomc[b, 1]
    for m in range(M):
        aux[:, 2 + m] = micc[b, k, m, 0]
        aux[:, 6 + m] = micc[b, k, m, 1]
    return {"signal": sig, "aux": aux}


def build_kernel():
    nc = bacc.Bacc("TRN2", target_bir_lowering=False, debug=False)
    sig_d = nc.declare_dram_parameter("signal", [NS, N], FP, isOutput=False)
    aux_d = nc.declare_dram_parameter("aux", [128, 12], FP, isOutput=False)
    cd = {
        name: nc.declare_dram_parameter(name, shape, FP, isOutput=False)
        for name, shape in CONST_SPECS
    }
    out_d = nc.declare_dram_parameter("out", [NBK, NQ], FP, isOutput=True)

    with tile.TileContext(nc) as tc:
        with ExitStack() as top:
            # small constants that live for the whole kernel
            cpool = top.enter_context(tc.tile_pool(name="consts", bufs=1))
            SMALL = {"a_re", "a_im", "b_re", "b_im", "b_im_neg", "e64_re", "e64_im",
                     "e64_im_neg", "e128_re", "e128_im_neg", "ident", "repl16"}
            ct = {}
            for name, shape in CONST_SPECS:
                if name in SMALL:
                    t = cpool.tile(shape, FP, tag=name, name=name)
                    nc.sync.dma_start(t[:], cd[name][:, :])
                    ct[name] = t
            aux = cpool.tile([128, 12], FP, tag="aux", name="aux")
            nc.sync.dma_start(aux[:], aux_d[:, :])
            bias_round = cpool.tile([128, 1], FP, tag="bias_round", name="bias_round")
            nc.vector.memset(bias_round[:], 512.0)
            bias_eps = cpool.tile([128, 1], FP, tag="bias_eps", name="bias_eps")
            nc.vector.memset(bias_eps[:], 1e-18)

            # gather inputs persist until the gather phase
            gpool = top.enter_context(tc.tile_pool(name="gin", bufs=1))
            gdata = [gpool.tile([128, NTAB], FP, tag=f"gdata{i}", name=f"gdata{i}") for i in range(NPA)]
            gidx = [gpool.tile([128, NTAB], I16, tag=f"gidx{i}", name=f"gidx{i}") for i in range(NPA)]

            # ---------------- geometry / index pipeline ----------------
            with ExitStack() as geo:
                gp = geo.enter_context(tc.tile_pool(name="geo", bufs=1))
                NF = NQ // 16  # 1024
                gxc = gp.tile([128, NF], FP, tag="gxc", name="gxc")
                nc.sync.dma_start(gxc[:], cd["gxc"][:, :])
                gyc = gp.tile([128, NF], FP, tag="gyc", name="gyc")
                nc.sync.dma_start(gyc[:], cd["gyc"][:, :])
                gx = gp.tile([128, NF], FP, tag="gx", name="gx")
                gy = gp.tile([128, NF], FP, tag="gy", name="gy")
                nc.vector.tensor_scalar_mul(gx[:], gxc[:], aux[:, 0:1])
                nc.vector.tensor_scalar_mul(gy[:], gyc[:], aux[:, 1:2])
                dist = [gp.tile([128, NF], FP, tag=f"dist{m}", name=f"dist{m}") for m in range(M)]
                negm = gp.tile([128, 8], FP, tag="negm", name="negm")
                nc.vector.tensor_scalar_mul(negm[:], aux[:, 2:10], -1.0)
                for m in range(M):
                    d2 = gp.tile([128, NF], FP, tag="d2", name="d2")
                    dy2 = gp.tile([128, NF], FP, tag="dy2", name="dy2")
                    nc.scalar.activation(d2[:], gx[:], ACTF.Square, bias=negm[:, m : m + 1])
                    nc.scalar.activation(
                        dy2[:], gy[:], ACTF.Square, bias=negm[:, 4 + m : 5 + m]
                    )
                    nc.vector.tensor_add(d2[:], d2[:], dy2[:])
                    nc.scalar.activation(dist[m][:], d2[:], ACTF.Sqrt)
                for i, (mi, mj) in enumerate(PAIRS):
                    ds = gp.tile([128, NF], FP, tag="ds", name="ds")
                    dsi = gp.tile([128, NF], I32, tag="dsi", name="dsi")
                    d16 = gp.tile([128, NF], I32, tag="d16", name="d16")
                    dl = gp.tile([128, NF], I32, tag="dl", name="dl")
                    nc.vector.tensor_sub(ds[:], dist[mi][:], dist[mj][:])
                    # dlocal+0.5 = ds*(SR/343) + 512.5 ; then clamp, truncate
                    nc.scalar.activation(
                        ds[:], ds[:], ACTF.Identity,
                        bias=bias_round[:, 0:1], scale=float(np.float32(SR) / np.float32(SPEED)),
                    )
                    nc.vector.tensor_scalar(
                        ds[:], ds[:], 0.0, 1023.0, op0=ALU.max, op1=ALU.min
                    )
                    nc.vector.tensor_copy(dsi[:], ds[:])
                    # permuted index d' = ((d*16) & 1023) + ((d*16) >> 10)
                    nc.vector.tensor_scalar(d16[:], dsi[:], 16, None, op0=ALU.mult)
                    nc.vector.tensor_scalar(dl[:], d16[:], 1023, None, op0=ALU.bitwise_and)
                    nc.vector.tensor_scalar(
                        d16[:], d16[:], 10, None, op0=ALU.logical_shift_right
                    )
                    nc.vector.tensor_tensor(gidx[i][:], d16[:], dl[:], op=ALU.add)

            # ------------- forward FFT + spectra + inverse (half/pair split) ----
            # Signal order is now s = m*NBK + bk (mics major), so half h of the
            # signal range covers mics {2h, 2h+1}. Front tiles are split per
            # half and the post-PHAT stages per pair so that pair 0's table
            # (and its gather) is ready long before the full front finishes.
            class MPool:
                def __init__(self, name, space="SBUF", bufs=1, side=None):
                    self.cm = tc.tile_pool(name=name, bufs=bufs, space=space, side=side)
                    self.pool = self.cm.__enter__()
                def tile(self, *a, **kw):
                    return self.pool.tile(*a, **kw)
                def close(self):
                    self.cm.__exit__(None, None, None)

            NSH = NS // 2  # 16 signals per half
            HC = NSH * P2  # 1024 cols per half

            # persistent-front pools (tiles span both halves; h-major order
            # below puts mic-{0,1} FFT + pair 0's whole chain + gather 0
            # ahead of the mic-{2,3} FFT in every engine queue)
            p_go = MPool("p_go", side="left", bufs=1)
            acc = p_go.tile([128, NQ], FP, tag="acc", name="acc")
            p_q = MPool("p_q", side="left", bufs=2)
            p_ytw = MPool("p_ytw", side="left")
            p_a = MPool("p_a", side="left")
            psA = MPool("psA", space="PSUM", bufs=1)
            twf_re = p_a.tile([128, P2], FP, name="twf_re")
            nc.sync.dma_start(twf_re[:], cd["twf_re"][:, 0:P2])
            twf_im = p_a.tile([128, P2], FP, name="twf_im")
            nc.sync.dma_start(twf_im[:], cd["twf_im"][:, 0:P2])
            twfr_b = twf_re[:].rearrange("p (o j) -> p o j", o=1).broadcast_to((128, 8, P2))
            twfi_b = twf_im[:].rearrange("p (o j) -> p o j", o=1).broadcast_to((128, 8, P2))
            p_yt = MPool("p_yt", side="right")

            p_z = MPool("p_z", side="left")
            p_uv = MPool("p_uv", side="right")
            u_mre, u_mim, v_mre, v_mim = {}, {}, {}, {}
            p_n = MPool("p_n", side="right", bufs=1)
            dw_res = p_n.tile([128, P2], FP, tag="dwr", name="dw_res")
            nc.sync.dma_start(dw_res[:], cd["dw_re"][:, 0:P2])
            dw_ims = p_n.tile([128, P2], FP, tag="dwi", name="dw_ims")
            nc.sync.dma_start(dw_ims[:], cd["dw_im"][:, 0:P2])
            dw_re3 = dw_res[:].rearrange("p (o j) -> p o j", o=1).broadcast_to((128, NSH, P2))
            dw_im3 = dw_ims[:].rearrange("p (o j) -> p o j", o=1).broadcast_to((128, NSH, P2))
            p_bk = MPool("p_bk", side="right", bufs=1)
            twi_res = p_bk.tile([128, P2], FP, tag="twir", name="twi_res")
            nc.sync.dma_start(twi_res[:], cd["twi_re"][:, 0:P2])
            twi_ims = p_bk.tile([128, P2], FP, tag="twii", name="twi_ims")
            nc.sync.dma_start(twi_ims[:], cd["twi_im"][:, 0:P2])
            twir_b = twi_res[:].rearrange("p (o t) -> p o t", o=1).broadcast_to((128, 8, P2))
            twii_b = twi_ims[:].rearrange("p (o t) -> p o t", o=1).broadcast_to((128, 8, P2))
            NQH = NQ // 2

            def mview(tiles, m):
                return tiles[m][:].rearrange("p (bk k) -> p bk k", bk=NBK)

            for h in range(2):

                # step A + twiddle for this half
                x2h = p_a.tile([128, NSH, P2], FP, tag="x2", name=f"x2{h}")
                nc.sync.dma_start(
                    x2h[:],
                    sig_d[h * NSH : (h + 1) * NSH, :].rearrange("s (p j) -> p s j", p=128),
                )
                x2f = x2h[:].rearrange("p s j -> p (s j)")
                z_lc_re, z_lc_im = {}, {}
                for lc in range(2):
                    sl = slice(lc * 512, (lc + 1) * 512)
                    yre = psA.tile([128, 512], FP, tag="t1", name="yre")
                    yim = psA.tile([128, 512], FP, tag="t2", name="yim")
                    nc.tensor.matmul(yre[:], ct["a_re"][:], x2f[:, sl])
                    nc.tensor.matmul(yim[:], ct["a_im"][:], x2f[:, sl])
                    tmp = p_a.tile([128, 512], FP, tag="twtmp", name="twtmp")
                    ytwr = p_ytw.tile([128, 512], FP, tag="ytw_re", name=f"ytw_re{h}{lc}")
                    ytwi = p_ytw.tile([128, 512], FP, tag="ytw_im", name=f"ytw_im{h}{lc}")
                    yre3 = yre[:].rearrange("p (s j) -> p s j", j=P2)
                    yim3 = yim[:].rearrange("p (s j) -> p s j", j=P2)
                    tmp3 = tmp[:].rearrange("p (s j) -> p s j", j=P2)
                    ytw_re3 = ytwr[:].rearrange("p (s j) -> p s j", j=P2)
                    ytw_im3 = ytwi[:].rearrange("p (s j) -> p s j", j=P2)
                    nc.vector.tensor_mul(tmp3, yre3, twfr_b)
                    nc.vector.tensor_mul(ytw_re3, yim3, twfi_b)
                    nc.vector.tensor_sub(ytw_re3, tmp3, ytw_re3)
                    nc.vector.tensor_mul(tmp3, yre3, twfi_b)
                    nc.vector.tensor_mul(ytw_im3, yim3, twfr_b)
                    nc.vector.tensor_add(ytw_im3, tmp3, ytw_im3)
                    # transpose + step B, one 4-signal chunk at a time
                    zre = psA.tile([128, 512], FP, tag="t1", name="zre")
                    zim = psA.tile([128, 512], FP, tag="t2", name="zim")
                    for c4 in range(2):
                        yTr = p_yt.tile([64, 512], FP, tag="ytTc_re", name=f"yTr{h}{lc}{c4}")
                        yTi = p_yt.tile([64, 512], FP, tag="ytTc_im", name=f"yTi{h}{lc}{c4}")
                        for srcv, dst in [(ytwr, yTr), (ytwi, yTi)]:
                            pt = psA.tile([64, 512], FP, tag="t3", name="ptr")
                            for k in range(4):
                                ls = c4 * 4 + k
                                nc.tensor.transpose(
                                    pt[:, k * 128 : (k + 1) * 128],
                                    srcv[:, ls * 64 : (ls + 1) * 64], ct["ident"][:],
                                )
                            nc.scalar.copy(dst[:], pt[:])
                        for col in range(4):
                            si = c4 * 4 + col
                            lre = yTr[:, col * 128 : (col + 1) * 128]
                            lim = yTi[:, col * 128 : (col + 1) * 128]
                            osl = slice(si * 64, (si + 1) * 64)
                            nc.tensor.matmul(zre[:, osl], lre, ct["b_re"][:], start=True, stop=False)
                            nc.tensor.matmul(zre[:, osl], lim, ct["b_im_neg"][:], start=False, stop=True)
                            nc.tensor.matmul(zim[:, osl], lre, ct["b_im"][:], start=True, stop=False)
                            nc.tensor.matmul(zim[:, osl], lim, ct["b_re"][:], start=False, stop=True)
                    z_lc_re[lc] = p_z.tile([128, 512], FP, tag="z_re", name=f"z_re{h}{lc}")
                    z_lc_im[lc] = p_z.tile([128, 512], FP, tag="z_im", name=f"z_im{h}{lc}")
                    nc.scalar.copy(z_lc_re[lc][:], zre[:])
                    nc.scalar.copy(z_lc_im[lc][:], zim[:])
                # PHAT + Dirichlet per mic (chunk ph <-> mic 2h+ph); V_3 unused
                for ph in range(2):
                    m = 2 * h + ph
                    nrm = p_n.tile([128, 512], FP, tag="nrm", name="nrm")
                    tmp2 = p_n.tile([128, 512], FP, tag="tmp2", name="tmp2")
                    nc.vector.tensor_mul(nrm[:], z_lc_re[ph][:], z_lc_re[ph][:])
                    nc.vector.tensor_mul(tmp2[:], z_lc_im[ph][:], z_lc_im[ph][:])
                    nc.vector.tensor_add(nrm[:], nrm[:], tmp2[:])
                    nc.scalar.activation(nrm[:], nrm[:], ACTF.Sqrt, bias=bias_eps[:, 0:1])
                    nc.vector.reciprocal(nrm[:], nrm[:])
                    u_mre[m] = p_uv.tile([128, 512], FP, tag=f"um_re{m % 2}", name=f"u_re_m{m}")
                    u_mim[m] = p_uv.tile([128, 512], FP, tag=f"um_im{m % 2}", name=f"u_im_m{m}")
                    nc.vector.tensor_mul(u_mre[m][:], z_lc_re[ph][:], nrm[:])
                    nc.vector.tensor_mul(u_mim[m][:], z_lc_im[ph][:], nrm[:])
                    if m < 3:
                        v_mre[m] = p_uv.tile([128, 512], FP, tag=f"vm_re{m}", name=f"v_re_m{m}")
                        v_mim[m] = p_uv.tile([128, 512], FP, tag=f"vm_im{m}", name=f"v_im_m{m}")
                        ur3 = u_mre[m][:].rearrange("p (s j) -> p s j", j=P2)
                        ui3 = u_mim[m][:].rearrange("p (s j) -> p s j", j=P2)
                        vr3 = v_mre[m][:].rearrange("p (s j) -> p s j", j=P2)
                        vi3 = v_mim[m][:].rearrange("p (s j) -> p s j", j=P2)
                        tm3 = tmp2[:].rearrange("p (s j) -> p s j", j=P2)
                        dw_re8 = dw_res[:].rearrange("p (o j) -> p o j", o=1).broadcast_to((128, 8, P2))
                        dw_im8 = dw_ims[:].rearrange("p (o j) -> p o j", o=1).broadcast_to((128, 8, P2))
                        nc.vector.tensor_mul(vr3, ur3, dw_re8)
                        nc.vector.tensor_mul(tm3, ui3, dw_im8)
                        nc.vector.tensor_sub(vr3, vr3, tm3)
                        nc.vector.tensor_mul(vi3, ur3, dw_im8)
                        nc.vector.tensor_mul(tm3, ui3, dw_re8)
                        nc.vector.tensor_add(vi3, vi3, tm3)

                # pairs available after this half, with full back chain+gather
                for i in ([0] if h == 0 else [1, 2, 3, 4, 5]):
                    mi, mj = PAIRS[i]
                    vi_re, vi_im = mview(v_mre, mi), mview(v_mim, mi)
                    uj_re, uj_im = mview(u_mre, mj), mview(u_mim, mj)
                    q_re_i = p_q.tile([128, NBK, P2], FP, tag="q_re", name=f"q_re{i}")
                    q_im_i = p_q.tile([128, NBK, P2], FP, tag="q_im", name=f"q_im{i}")
                    tq = p_q.tile([128, NBK, P2], FP, tag="qtmp", name="qtmp")
                    nc.vector.tensor_mul(q_re_i[:], vi_re, uj_re)
                    nc.vector.tensor_mul(tq[:], vi_im, uj_im)
                    nc.vector.tensor_add(q_re_i[:], q_re_i[:], tq[:])
                    nc.vector.tensor_mul(q_im_i[:], vi_im, uj_re)
                    nc.vector.tensor_mul(tq[:], vi_re, uj_im)
                    nc.vector.tensor_sub(q_im_i[:], q_im_i[:], tq[:])
                    # transpose Q_i -> Qt + inverse inner, per bk-half
                    qf_re = q_re_i[:].rearrange("p bk k -> p (bk k)")
                    qf_im = q_im_i[:].rearrange("p bk k -> p (bk k)")
                    ire = psA.tile([128, 512], FP, tag="t6", name="ire")
                    iim = psA.tile([128, 512], FP, tag="t7", name="iim")
                    for b4 in range(2):
                        qt_re = p_bk.tile([64, 512], FP, tag="qt_re", name=f"qt_re{i}_{b4}")
                        qt_im = p_bk.tile([64, 512], FP, tag="qt_im", name=f"qt_im{i}_{b4}")
                        for srcf, dst in [(qf_re, qt_re), (qf_im, qt_im)]:
                            pt = psA.tile([64, 512], FP, tag="t3", name="ptq")
                            for k in range(4):
                                bkk = b4 * 4 + k
                                nc.tensor.transpose(
                                    pt[:, k * 128 : (k + 1) * 128],
                                    srcf[:, bkk * 64 : (bkk + 1) * 64], ct["ident"][:],
                                )
                            nc.scalar.copy(dst[:], pt[:])
                        for pi in range(4):
                            bkpi = b4 * 4 + pi
                            lre = qt_re[:, pi * 128 : (pi + 1) * 128]
                            lim = qt_im[:, pi * 128 : (pi + 1) * 128]
                            osl = slice(bkpi * 64, (bkpi + 1) * 64)
                            nc.tensor.matmul(ire[:, osl], lre, ct["e64_re"][:], start=True, stop=False)
                            nc.tensor.matmul(ire[:, osl], lim, ct["e64_im_neg"][:], start=False, stop=True)
                            nc.tensor.matmul(iim[:, osl], lre, ct["e64_im"][:], start=True, stop=False)
                            nc.tensor.matmul(iim[:, osl], lim, ct["e64_re"][:], start=False, stop=True)
                    in_re = p_bk.tile([128, 512], FP, tag="in_re", name=f"in_re{i}")
                    in_im = p_bk.tile([128, 512], FP, tag="in_im", name=f"in_im{i}")
                    t_a = p_bk.tile([128, 512], FP, tag="t_a", name="t_a")
                    ire3 = ire[:].rearrange("p (r t) -> p r t", t=P2)
                    iim3 = iim[:].rearrange("p (r t) -> p r t", t=P2)
                    ta3 = t_a[:].rearrange("p (r t) -> p r t", t=P2)
                    inre3 = in_re[:].rearrange("p (r t) -> p r t", t=P2)
                    inim3 = in_im[:].rearrange("p (r t) -> p r t", t=P2)
                    nc.vector.tensor_mul(ta3, ire3, twir_b)
                    nc.vector.tensor_mul(inre3, iim3, twii_b)
                    nc.vector.tensor_sub(inre3, ta3, inre3)
                    nc.vector.tensor_mul(ta3, ire3, twii_b)
                    nc.vector.tensor_mul(inim3, iim3, twir_b)
                    nc.vector.tensor_add(inim3, inim3, ta3)
                    # inverse outer -> tt_i [NT1, NBK*P2]
                    ot = psA.tile([NT1, 512], FP, tag="t4", name="ot")
                    nc.tensor.matmul(ot[:], ct["e128_re"][:], in_re[:], start=True, stop=False)
                    nc.tensor.matmul(ot[:], ct["e128_im_neg"][:], in_im[:], start=False, stop=True)
                    tt_i = p_bk.tile([NT1, 512], FP, tag="tt", name=f"tt{i}")
                    nc.scalar.copy(tt_i[:], ot[:])
                    # per-pair table: [NBK, NTAB] via 64 small transposes
                    tt3 = tt_i[:].rearrange("a (bk t) -> a bk t", bk=NBK)
                    ptab = psA.tile([NBK, NTAB], FP, tag="t5", name="ptab")
                    for t2v in range(P2):
                        nc.tensor.transpose(
                            ptab[:, t2v * NT1 : (t2v + 1) * NT1],
                            tt3[:, :, t2v : t2v + 1],
                            ct["ident"][0:NT1, 0:NT1],
                        )
                    tabs_i = p_bk.tile([NBK, NTAB], FP, tag="tabs", name=f"tabs{i}")
                    nc.scalar.copy(tabs_i[:], ptab[:])
                    # replicate each bk row across its 16 partitions
                    for halfc in range(2):
                        sl = slice(halfc * 512, (halfc + 1) * 512)
                        prep = psA.tile([128, 512], FP, tag="t6", name="prep")
                        nc.tensor.matmul(prep[:], ct["repl16"][:], tabs_i[:, sl])
                        nc.scalar.copy(gdata[i][:, sl], prep[:])
                    # gather in two halves (keeps the bounce buffer at 32KB)
                    for gh in range(2):
                        hsl = slice(gh * NQH, (gh + 1) * NQH)
                        gout = p_go.tile([128, NQH], FP, tag="gout", name="gout")
                        if i == 0 and gh == 0:
                            nc.gpsimd.load_library(library_config.ap_gather)
                        nc.gpsimd.ap_gather(
                            gout[:], gdata[i][:], gidx[i][:, gh * 512 : (gh + 1) * 512],
                            channels=128, num_elems=NTAB, d=1, num_idxs=NQH,
                        )
                        if i == 0:
                            nc.vector.tensor_copy(acc[:, hsl], gout[:])
                        else:
                            nc.vector.tensor_add(acc[:, hsl], acc[:, hsl], gout[:])

            p_bk.close()
            p_n.close()
            p_uv.close()
            p_yt.close()
            psA.close()
            p_z.close()
            p_a.close()
            p_ytw.close()

            # ---------------- normalize + out ----------------
            with ExitStack() as gph:
                op = gph.enter_context(tc.tile_pool(name="gout2", bufs=1))
                psn = gph.enter_context(tc.tile_pool(name="psn", bufs=1, space="PSUM"))
                m128 = op.tile([128, 1], FP, tag="m128", name="m128")
                nc.vector.tensor_reduce(
                    m128[:], acc[:], axis=mybir.AxisListType.X, op=ALU.max
                )
                mt = psn.tile([1, 128], FP, tag="mt", name="mt")
                nc.tensor.transpose(mt[:], m128[:], ct["ident"][:])
                mts = op.tile([1, 128], FP, tag="mts", name="mts")
                nc.scalar.copy(mts[:], mt[:])
                mg = op.tile([1, NBK], FP, tag="mg", name="mg")
                nc.vector.tensor_reduce(
                    mg[:],
                    mts[:].rearrange("a (g r) -> a g r", r=16),
                    axis=mybir.AxisListType.X, op=ALU.max,
                )
                nc.vector.reciprocal(mg[:], mg[:])
                mgt = psn.tile([NBK, 1], FP, tag="mgt", name="mgt")
                nc.tensor.transpose(mgt[:], mg[:], ct["ident"][0:1, 0:1])
                mgs = op.tile([NBK, 1], FP, tag="mgs", name="mgs")
                nc.scalar.copy(mgs[:], mgt[:])
                scp = psn.tile([128, 1], FP, tag="scp", name="scp")
                nc.tensor.matmul(scp[:], ct["repl16"][:], mgs[:])
                sc = op.tile([128, 1], FP, tag="sc", name="sc")
                nc.scalar.copy(sc[:], scp[:])
                for ch in range(NQ // 2048):
                    sl = slice(ch * 2048, (ch + 1) * 2048)
                    grids = op.tile([128, 2048], FP, tag="grids", name="grids")
                    nc.scalar.activation(
                        grids[:], acc[:, sl], ACTF.Copy, scale=sc[:, 0:1]
                    )
                    gv = grids[:].rearrange("(g r) q -> g r q", r=16)[:, 0, :]
                    nc.sync.dma_start(out_d[:, sl], gv)
            p_q.close()
            p_go.close()

    nc.compile()
    return nc


_NC_CACHE = {}


def kernel(signal, mic_coordinates, room_dims):
    signal = np.ascontiguousarray(np.asarray(signal, dtype=np.float32))
    mic_coordinates = np.ascontiguousarray(np.asarray(mic_coordinates, dtype=np.float32))
    room_dims = np.ascontiguousarray(np.asarray(room_dims, dtype=np.float32))
    if "nc" not in _NC_CACHE:
        _NC_CACHE["nc"] = build_kernel()
        _NC_CACHE["consts"] = make_consts()
    nc = _NC_CACHE["nc"]
    consts = _NC_CACHE["consts"]
    in_maps = []
    for core in range(8):
        m = shard_core(signal, mic_coordinates, room_dims, core)
        m.update(consts)
        in_maps.append(m)
    res = run_bass_kernel_spmd(nc, in_maps, core_ids=list(range(8)), trace=False)
    outs = [res.results[c]["out"].reshape(NB * K, NQ) for c in range(8)]
    return np.concatenate(outs, axis=0).reshape(B, K, NQ).astype(np.float32)



# revision 5
# speedup vs baseline: 1.0449x; 1.0124x over previous
"""GCC-PHAT spatial likelihood grid kernel for Trainium2 (8 NeuronCores).

Self-contained: kernel(**inputs) -> np.ndarray. Shards the batch over the 8
cores (pure data parallel), runs a Bass/Tile kernel per core, gathers.
"""

import json

import numpy as np
from contextlib import ExitStack

import concourse.bass as bass
import concourse.bacc as bacc
import concourse.mybir as mybir
from concourse import tile, library_config
from concourse.bass_utils import run_bass_kernel_spmd
from concourse.tile import TileContext
from bass_rust import ScopedClock

# ---------------------------------------------------------------------------
# Workaround 1: this walrus build allows at most one semaphore wait per
# instruction. Post-process the BIR JSON: excess waits move onto NoOps
# inserted just before the offending instruction (same engine, so ordering
# is preserved).
_uid = [0]


def _fix_module(m):
    for f in m.get("functions", []):
        for bb in f.get("blocks", []):
            insts = bb.get("instructions")
            if not insts:
                continue
            out = []
            changed = False
            for ins in insts:
                si = ins.get("sync_info")
                ow = (si or {}).get("on_wait") or []
                if len(ow) > 1:
                    changed = True
                    for w in ow[1:]:
                        _uid[0] += 1
                        out.append({
                            "engine": ins["engine"], "ins": [], "outs": [],
                            "name": f"WFix-{_uid[0]}", "opcode": "NoOp",
                            "sync_info": {"on_update": [], "on_wait": [w]},
                        })
                    si["on_wait"] = ow[:1]
                out.append(ins)
            if changed:
                bb["instructions"] = out
    return m


_orig_to_json_bytes = bass.Bass.to_json_bytes


def _to_json_bytes(self):
    return json.dumps(_fix_module(json.loads(_orig_to_json_bytes(self)))).encode()


bass.Bass.to_json_bytes = _to_json_bytes

# Workaround 2: the TileContext tail Drain gets zero wait slots here; move
# the end-of-kernel waits onto SP NoOps emitted before the drain.


def _drain_and_barrier(self, tick_clock, wait_clock):
    nc = self.nc
    first_nop = nc.sync.nop()
    wait_clock.add_sem_waits(first_nop.ins, ScopedClock({None: tick_clock.global_clock}))
    si = first_nop.ins.sync_info
    if si is not None and len(si.on_wait) > 1:
        waits = list(si.on_wait)
        first_nop.ins.sync_info = mybir.SyncInfo(
            on_wait=waits[:1], on_update=list(si.on_update)
        )
        for w in waits[1:]:
            nop = nc.sync.nop()
            nop.ins.sync_info = mybir.SyncInfo(on_wait=[w], on_update=[])
    nc.sync.drain()
    nc.all_engine_barrier()
    popped = nc._tile_sem_poison_stack.pop()
    assert popped is self._sem_poison
    nc.clear_and_free_semaphores(list(self.sems.allocated().values()))
    nc.all_engine_barrier()


TileContext._drain_and_barrier = _drain_and_barrier

# ---------------------------------------------------------------------------
FP = mybir.dt.float32
I32 = mybir.dt.int32
I16 = mybir.dt.int16
ALU = mybir.AluOpType
ACTF = mybir.ActivationFunctionType

SR = 16000.0
SPEED = 343.0
G = 128
B, K, M, N = 16, 4, 4, 8192
P1, P2 = 128, 64  # N = P1*P2 ; n = p*64 + j ; bin k = k2*128 + k1
PAIRS = [(0, 1), (0, 2), (0, 3), (1, 2), (1, 3), (2, 3)]
NPA = 6
NB = 2
NBK = NB * K  # 8
NS = NBK * M  # 32
NPAIR = NBK * NPA  # 48
T1LO, T1HI = 56, 72
NT1 = T1HI - T1LO  # 16
NTAB = NT1 * P2  # 1024
NQ = G * G  # 16384
G2HI = G // 16  # 8


def make_consts():
    c = {}
    k1 = np.arange(P1)
    j = np.arange(P2)
    k2 = np.arange(P2)
    t2 = np.arange(P2)
    p = np.arange(P1)

    ang = 2 * np.pi * np.outer(p, k1) / P1
    c["a_re"] = np.cos(ang).astype(np.float32)  # [p, k1]
    c["a_im"] = (-np.sin(ang)).astype(np.float32)

    angt = 2 * np.pi * np.outer(k1, j) / N  # fwd twiddle e^{-i...}
    c["twf_re"] = np.tile(np.cos(angt).astype(np.float32), (1, NS))  # [128, 2048]
    c["twf_im"] = np.tile((-np.sin(angt)).astype(np.float32), (1, NS))

    angb = 2 * np.pi * np.outer(j, k2) / P2  # W64 = e^{-i...}
    c["b_re"] = np.cos(angb).astype(np.float32)  # [j, k2]
    c["b_im"] = (-np.sin(angb)).astype(np.float32)
    c["b_im_neg"] = -c["b_im"]

    kk = k1[:, None] + P1 * k2[None, :]  # [k1, k2] bin index
    o = np.arange(-5, 5)
    D = np.exp(2j * np.pi * kk[..., None] * o / N).sum(-1)
    DW = D * ((-1.0) ** kk) / N
    c["dw_re"] = np.tile(np.ascontiguousarray(DW.real).astype(np.float32), (1, NS))
    c["dw_im"] = np.tile(np.ascontiguousarray(DW.imag).astype(np.float32), (1, NS))

    angi = 2 * np.pi * np.outer(k2, t2) / P2  # E64 = e^{+i...}
    c["e64_re"] = np.cos(angi).astype(np.float32)  # [k2, t2]
    c["e64_im"] = np.sin(angi).astype(np.float32)
    c["e64_im_neg"] = -c["e64_im"]

    angti = 2 * np.pi * np.outer(k1, t2) / N  # inv twiddle e^{+i...}
    c["twi_re"] = np.tile(np.cos(angti).astype(np.float32), (1, NPAIR))  # [128, 3072]
    c["twi_im"] = np.tile(np.sin(angti).astype(np.float32), (1, NPAIR))

    t1 = np.arange(T1LO, T1HI)
    ango = 2 * np.pi * np.outer(k1, t1) / P1  # E128 = e^{+i...}
    c["e128_re"] = np.cos(ango).astype(np.float32)  # [k1, 16]
    c["e128_im_neg"] = (-np.sin(ango)).astype(np.float32)

    c["ident"] = np.eye(128, dtype=np.float32)

    # group-broadcast selector: lhsT [8, 128]; out row p <- in row p//16
    r16 = np.zeros((NBK, 128), np.float32)
    for g in range(NBK):
        r16[g, g * 16 : (g + 1) * 16] = 1.0
    c["repl16"] = r16

    # grid parameter fields (wrapped layout): partition = (bk, g2lo), free = (g1, g2hi)
    t_lin = np.linspace(0.0, 1.0, G).astype(np.float32)
    g2lo = (p % 16)[:, None, None]
    g1 = np.arange(G)[None, :, None]
    g2hi = np.arange(G2HI)[None, None, :]
    gxc = np.broadcast_to(t_lin[g1], (128, G, G2HI))
    gyc = t_lin[(g2hi * 16 + g2lo)] * np.ones((128, G, G2HI), np.float32)
    c["gxc"] = np.ascontiguousarray(gxc.reshape(128, NQ // 16), np.float32)
    c["gyc"] = np.ascontiguousarray(gyc.reshape(128, NQ // 16), np.float32)
    return c


CONST_SPECS = [
    ("a_re", [128, 128]), ("a_im", [128, 128]),
    ("twf_re", [128, 64 * NS]), ("twf_im", [128, 64 * NS]),
    ("b_re", [64, 64]), ("b_im", [64, 64]), ("b_im_neg", [64, 64]),
    ("dw_re", [128, 64 * NS]), ("dw_im", [128, 64 * NS]),
    ("e64_re", [64, 64]), ("e64_im", [64, 64]), ("e64_im_neg", [64, 64]),
    ("twi_re", [128, 64 * NPAIR]), ("twi_im", [128, 64 * NPAIR]),
    ("e128_re", [128, NT1]), ("e128_im_neg", [128, NT1]),
    ("ident", [128, 128]),
    ("repl16", [NBK, 128]),
    ("gxc", [128, NQ // 16]), ("gyc", [128, NQ // 16]),
]


def shard_core(signal, mic, room, core):
    b0 = core * NB
    sig = np.ascontiguousarray(signal[b0 : b0 + NB].transpose(2, 0, 1, 3).reshape(NS, N))
    micc = mic[b0 : b0 + NB]
    roomc = room[b0 : b0 + NB]
    pp = np.arange(128)
    bk = pp // 16
    b = bk // K
    k = bk % K
    aux = np.zeros((128, 12), np.float32)
    aux[:, 0] = roomc[b, 0]
    aux[:, 1] = roomc[b, 1]
    for m in range(M):
        aux[:, 2 + m] = micc[b, k, m, 0]
        aux[:, 6 + m] = micc[b, k, m, 1]
    return {"signal": sig, "aux": aux}


def build_kernel():
    nc = bacc.Bacc("TRN2", target_bir_lowering=False, debug=False)
    sig_d = nc.declare_dram_parameter("signal", [NS, N], FP, isOutput=False)
    aux_d = nc.declare_dram_parameter("aux", [128, 12], FP, isOutput=False)
    cd = {
        name: nc.declare_dram_parameter(name, shape, FP, isOutput=False)
        for name, shape in CONST_SPECS
    }
    out_d = nc.declare_dram_parameter("out", [NBK, NQ], FP, isOutput=True)

    with tile.TileContext(nc) as tc:
        with ExitStack() as top:
            # small constants that live for the whole kernel
            cpool = top.enter_context(tc.tile_pool(name="consts", bufs=1))
            SMALL = {"a_re", "a_im", "b_re", "b_im", "b_im_neg", "e64_re", "e64_im",
                     "e64_im_neg", "e128_re", "e128_im_neg", "ident", "repl16"}
            ct = {}
            for name, shape in CONST_SPECS:
                if name in SMALL:
                    t = cpool.tile(shape, FP, tag=name, name=name)
                    nc.sync.dma_start(t[:], cd[name][:, :])
                    ct[name] = t
            aux = cpool.tile([128, 12], FP, tag="aux", name="aux")
            nc.sync.dma_start(aux[:], aux_d[:, :])
            bias_round = cpool.tile([128, 1], FP, tag="bias_round", name="bias_round")
            nc.vector.memset(bias_round[:], 512.0)
            bias_eps = cpool.tile([128, 1], FP, tag="bias_eps", name="bias_eps")
            nc.vector.memset(bias_eps[:], 1e-18)

            # gather inputs persist until the gather phase
            gpool = top.enter_context(tc.tile_pool(name="gin", bufs=1))
            gdata = [gpool.tile([128, NTAB], FP, tag=f"gdata{i}", name=f"gdata{i}") for i in range(NPA)]
            gidx = [gpool.tile([128, NTAB], I16, tag=f"gidx{i}", name=f"gidx{i}") for i in range(NPA)]

            # ---------------- geometry / index pipeline ----------------
            with ExitStack() as geo:
                gp = geo.enter_context(tc.tile_pool(name="geo", bufs=1))
                NF = NQ // 16  # 1024
                gxc = gp.tile([128, NF], FP, tag="gxc", name="gxc")
                nc.sync.dma_start(gxc[:], cd["gxc"][:, :])
                gyc = gp.tile([128, NF], FP, tag="gyc", name="gyc")
                nc.sync.dma_start(gyc[:], cd["gyc"][:, :])
                gx = gp.tile([128, NF], FP, tag="gx", name="gx")
                gy = gp.tile([128, NF], FP, tag="gy", name="gy")
                nc.vector.tensor_scalar_mul(gx[:], gxc[:], aux[:, 0:1])
                nc.vector.tensor_scalar_mul(gy[:], gyc[:], aux[:, 1:2])
                dist = [gp.tile([128, NF], FP, tag=f"dist{m}", name=f"dist{m}") for m in range(M)]
                negm = gp.tile([128, 8], FP, tag="negm", name="negm")
                nc.vector.tensor_scalar_mul(negm[:], aux[:, 2:10], -1.0)
                for m in range(M):
                    d2 = gp.tile([128, NF], FP, tag="d2", name="d2")
                    dy2 = gp.tile([128, NF], FP, tag="dy2", name="dy2")
                    nc.scalar.activation(d2[:], gx[:], ACTF.Square, bias=negm[:, m : m + 1])
                    nc.scalar.activation(
                        dy2[:], gy[:], ACTF.Square, bias=negm[:, 4 + m : 5 + m]
                    )
                    nc.vector.tensor_add(d2[:], d2[:], dy2[:])
                    nc.scalar.activation(dist[m][:], d2[:], ACTF.Sqrt)
                for i, (mi, mj) in enumerate(PAIRS):
                    ds = gp.tile([128, NF], FP, tag="ds", name="ds")
                    dsi = gp.tile([128, NF], I32, tag="dsi", name="dsi")
                    d16 = gp.tile([128, NF], I32, tag="d16", name="d16")
                    dl = gp.tile([128, NF], I32, tag="dl", name="dl")
                    nc.vector.tensor_sub(ds[:], dist[mi][:], dist[mj][:])
                    # dlocal+0.5 = ds*(SR/343) + 512.5 ; then clamp, truncate
                    nc.scalar.activation(
                        ds[:], ds[:], ACTF.Identity,
                        bias=bias_round[:, 0:1], scale=float(np.float32(SR) / np.float32(SPEED)),
                    )
                    nc.vector.tensor_scalar(
                        ds[:], ds[:], 0.0, 1023.0, op0=ALU.max, op1=ALU.min
                    )
                    nc.vector.tensor_copy(dsi[:], ds[:])
                    # permuted index d' = ((d*16) & 1023) + ((d*16) >> 10)
                    nc.vector.tensor_scalar(d16[:], dsi[:], 16, None, op0=ALU.mult)
                    nc.vector.tensor_scalar(dl[:], d16[:], 1023, None, op0=ALU.bitwise_and)
                    nc.vector.tensor_scalar(
                        d16[:], d16[:], 10, None, op0=ALU.logical_shift_right
                    )
                    nc.vector.tensor_tensor(gidx[i][:], d16[:], dl[:], op=ALU.add)

            # ------------- forward FFT + spectra + inverse (half/pair split) ----
            # Signal order is now s = m*NBK + bk (mics major), so half h of the
            # signal range covers mics {2h, 2h+1}. Front tiles are split per
            # half and the post-PHAT stages per pair so that pair 0's table
            # (and its gather) is ready long before the full front finishes.
            class MPool:
                def __init__(self, name, space="SBUF", bufs=1, side=None):
                    self.cm = tc.tile_pool(name=name, bufs=bufs, space=space, side=side)
                    self.pool = self.cm.__enter__()
                def tile(self, *a, **kw):
                    return self.pool.tile(*a, **kw)
                def close(self):
                    self.cm.__exit__(None, None, None)

            NSH = NS // 2  # 16 signals per half
            HC = NSH * P2  # 1024 cols per half

            # persistent-front pools (tiles span both halves; h-major order
            # below puts mic-{0,1} FFT + pair 0's whole chain + gather 0
            # ahead of the mic-{2,3} FFT in every engine queue)
            p_go = MPool("p_go", side="left", bufs=1)
            acc = p_go.tile([128, NQ], FP, tag="acc", name="acc")
            p_q = MPool("p_q", side="left", bufs=2)
            p_gb = MPool("p_gb", side="left", bufs=1)
            p_ytw = MPool("p_ytw", side="left")
            p_a = MPool("p_a", side="left")
            psA = MPool("psA", space="PSUM", bufs=1)
            twf_re = p_a.tile([128, P2], FP, name="twf_re")
            nc.sync.dma_start(twf_re[:], cd["twf_re"][:, 0:P2])
            twf_im = p_a.tile([128, P2], FP, name="twf_im")
            nc.sync.dma_start(twf_im[:], cd["twf_im"][:, 0:P2])
            twfr_b = twf_re[:].rearrange("p (o j) -> p o j", o=1).broadcast_to((128, 8, P2))
            twfi_b = twf_im[:].rearrange("p (o j) -> p o j", o=1).broadcast_to((128, 8, P2))
            p_yt = MPool("p_yt", side="right")

            p_z = MPool("p_z", side="left")
            p_uv = MPool("p_uv", side="right")
            u_mre, u_mim, v_mre, v_mim = {}, {}, {}, {}
            p_n = MPool("p_n", side="right", bufs=1)
            dw_res = p_n.tile([128, P2], FP, tag="dwr", name="dw_res")
            nc.sync.dma_start(dw_res[:], cd["dw_re"][:, 0:P2])
            dw_ims = p_n.tile([128, P2], FP, tag="dwi", name="dw_ims")
            nc.sync.dma_start(dw_ims[:], cd["dw_im"][:, 0:P2])
            dw_re3 = dw_res[:].rearrange("p (o j) -> p o j", o=1).broadcast_to((128, NSH, P2))
            dw_im3 = dw_ims[:].rearrange("p (o j) -> p o j", o=1).broadcast_to((128, NSH, P2))
            p_bk = MPool("p_bk", side="right", bufs=1)
            twi_res = p_bk.tile([128, P2], FP, tag="twir", name="twi_res")
            nc.sync.dma_start(twi_res[:], cd["twi_re"][:, 0:P2])
            twi_ims = p_bk.tile([128, P2], FP, tag="twii", name="twi_ims")
            nc.sync.dma_start(twi_ims[:], cd["twi_im"][:, 0:P2])
            twir_b = twi_res[:].rearrange("p (o t) -> p o t", o=1).broadcast_to((128, 8, P2))
            twii_b = twi_ims[:].rearrange("p (o t) -> p o t", o=1).broadcast_to((128, 8, P2))
            NQH = NQ // 2
            NQQ = NQ // 4

            def mview(tiles, m):
                return tiles[m][:].rearrange("p (bk k) -> p bk k", bk=NBK)

            for h in range(2):

                # step A + twiddle for this half
                x2h = p_a.tile([128, NSH, P2], FP, tag="x2", name=f"x2{h}")
                nc.sync.dma_start(
                    x2h[:],
                    sig_d[h * NSH : (h + 1) * NSH, :].rearrange("s (p j) -> p s j", p=128),
                )
                x2f = x2h[:].rearrange("p s j -> p (s j)")
                z_lc_re, z_lc_im = {}, {}
                for lc in range(2):
                    sl = slice(lc * 512, (lc + 1) * 512)
                    yre = psA.tile([128, 512], FP, tag="t1", name="yre")
                    yim = psA.tile([128, 512], FP, tag="t2", name="yim")
                    nc.tensor.matmul(yre[:], ct["a_re"][:], x2f[:, sl])
                    nc.tensor.matmul(yim[:], ct["a_im"][:], x2f[:, sl])
                    tmp = p_a.tile([128, 512], FP, tag="twtmp", name="twtmp")
                    ytwr = p_ytw.tile([128, 512], FP, tag="ytw_re", name=f"ytw_re{h}{lc}")
                    ytwi = p_ytw.tile([128, 512], FP, tag="ytw_im", name=f"ytw_im{h}{lc}")
                    yre3 = yre[:].rearrange("p (s j) -> p s j", j=P2)
                    yim3 = yim[:].rearrange("p (s j) -> p s j", j=P2)
                    tmp3 = tmp[:].rearrange("p (s j) -> p s j", j=P2)
                    ytw_re3 = ytwr[:].rearrange("p (s j) -> p s j", j=P2)
                    ytw_im3 = ytwi[:].rearrange("p (s j) -> p s j", j=P2)
                    nc.vector.tensor_mul(tmp3, yre3, twfr_b)
                    nc.vector.tensor_mul(ytw_re3, yim3, twfi_b)
                    nc.vector.tensor_sub(ytw_re3, tmp3, ytw_re3)
                    nc.vector.tensor_mul(tmp3, yre3, twfi_b)
                    nc.vector.tensor_mul(ytw_im3, yim3, twfr_b)
                    nc.vector.tensor_add(ytw_im3, tmp3, ytw_im3)
                    # transpose + step B, one 4-signal chunk at a time
                    zre = psA.tile([128, 512], FP, tag="t1", name="zre")
                    zim = psA.tile([128, 512], FP, tag="t2", name="zim")
                    for c4 in range(2):
                        yTr = p_yt.tile([64, 512], FP, tag="ytTc_re", name=f"yTr{h}{lc}{c4}")
                        yTi = p_yt.tile([64, 512], FP, tag="ytTc_im", name=f"yTi{h}{lc}{c4}")
                        for srcv, dst in [(ytwr, yTr), (ytwi, yTi)]:
                            pt = psA.tile([64, 512], FP, tag="t3", name="ptr")
                            for k in range(4):
                                ls = c4 * 4 + k
                                nc.tensor.transpose(
                                    pt[:, k * 128 : (k + 1) * 128],
                                    srcv[:, ls * 64 : (ls + 1) * 64], ct["ident"][:],
                                )
                            nc.scalar.copy(dst[:], pt[:])
                        for col in range(4):
                            si = c4 * 4 + col
                            lre = yTr[:, col * 128 : (col + 1) * 128]
                            lim = yTi[:, col * 128 : (col + 1) * 128]
                            osl = slice(si * 64, (si + 1) * 64)
                            nc.tensor.matmul(zre[:, osl], lre, ct["b_re"][:], start=True, stop=False)
                            nc.tensor.matmul(zre[:, osl], lim, ct["b_im_neg"][:], start=False, stop=True)
                            nc.tensor.matmul(zim[:, osl], lre, ct["b_im"][:], start=True, stop=False)
                            nc.tensor.matmul(zim[:, osl], lim, ct["b_re"][:], start=False, stop=True)
                    z_lc_re[lc] = p_z.tile([128, 512], FP, tag="z_re", name=f"z_re{h}{lc}")
                    z_lc_im[lc] = p_z.tile([128, 512], FP, tag="z_im", name=f"z_im{h}{lc}")
                    nc.scalar.copy(z_lc_re[lc][:], zre[:])
                    nc.scalar.copy(z_lc_im[lc][:], zim[:])
                # PHAT + Dirichlet per mic (chunk ph <-> mic 2h+ph); V_3 unused
                for ph in range(2):
                    m = 2 * h + ph
                    nrm = p_n.tile([128, 512], FP, tag="nrm", name="nrm")
                    tmp2 = p_n.tile([128, 512], FP, tag="tmp2", name="tmp2")
                    nc.vector.tensor_mul(nrm[:], z_lc_re[ph][:], z_lc_re[ph][:])
                    nc.vector.tensor_mul(tmp2[:], z_lc_im[ph][:], z_lc_im[ph][:])
                    nc.vector.tensor_add(nrm[:], nrm[:], tmp2[:])
                    nc.scalar.activation(nrm[:], nrm[:], ACTF.Sqrt, bias=bias_eps[:, 0:1])
                    nc.vector.reciprocal(nrm[:], nrm[:])
                    u_mre[m] = p_uv.tile([128, 512], FP, tag=f"um_re{m % 2}", name=f"u_re_m{m}")
                    u_mim[m] = p_uv.tile([128, 512], FP, tag=f"um_im{m % 2}", name=f"u_im_m{m}")
                    nc.vector.tensor_mul(u_mre[m][:], z_lc_re[ph][:], nrm[:])
                    nc.vector.tensor_mul(u_mim[m][:], z_lc_im[ph][:], nrm[:])
                    if m < 3:
                        v_mre[m] = p_uv.tile([128, 512], FP, tag=f"vm_re{m}", name=f"v_re_m{m}")
                        v_mim[m] = p_uv.tile([128, 512], FP, tag=f"vm_im{m}", name=f"v_im_m{m}")
                        ur3 = u_mre[m][:].rearrange("p (s j) -> p s j", j=P2)
                        ui3 = u_mim[m][:].rearrange("p (s j) -> p s j", j=P2)
                        vr3 = v_mre[m][:].rearrange("p (s j) -> p s j", j=P2)
                        vi3 = v_mim[m][:].rearrange("p (s j) -> p s j", j=P2)
                        tm3 = tmp2[:].rearrange("p (s j) -> p s j", j=P2)
                        dw_re8 = dw_res[:].rearrange("p (o j) -> p o j", o=1).broadcast_to((128, 8, P2))
                        dw_im8 = dw_ims[:].rearrange("p (o j) -> p o j", o=1).broadcast_to((128, 8, P2))
                        nc.vector.tensor_mul(vr3, ur3, dw_re8)
                        nc.vector.tensor_mul(tm3, ui3, dw_im8)
                        nc.vector.tensor_sub(vr3, vr3, tm3)
                        nc.vector.tensor_mul(vi3, ur3, dw_im8)
                        nc.vector.tensor_mul(tm3, ui3, dw_re8)
                        nc.vector.tensor_add(vi3, vi3, tm3)

                # pairs available after this half, with full back chain+gather
                for i in ([0] if h == 0 else [1, 2, 3, 4, 5]):
                    mi, mj = PAIRS[i]
                    vi_re, vi_im = mview(v_mre, mi), mview(v_mim, mi)
                    uj_re, uj_im = mview(u_mre, mj), mview(u_mim, mj)
                    q_re_i = p_q.tile([128, NBK, P2], FP, tag="q_re", name=f"q_re{i}")
                    q_im_i = p_q.tile([128, NBK, P2], FP, tag="q_im", name=f"q_im{i}")
                    tq = p_q.tile([128, NBK, P2], FP, tag="qtmp", name="qtmp")
                    nc.vector.tensor_mul(q_re_i[:], vi_re, uj_re)
                    nc.vector.tensor_mul(tq[:], vi_im, uj_im)
                    nc.vector.tensor_add(q_re_i[:], q_re_i[:], tq[:])
                    nc.vector.tensor_mul(q_im_i[:], vi_im, uj_re)
                    nc.vector.tensor_mul(tq[:], vi_re, uj_im)
                    nc.vector.tensor_sub(q_im_i[:], q_im_i[:], tq[:])
                    # transpose Q_i -> Qt + inverse inner, per bk-half
                    qf_re = q_re_i[:].rearrange("p bk k -> p (bk k)")
                    qf_im = q_im_i[:].rearrange("p bk k -> p (bk k)")
                    ire = psA.tile([128, 512], FP, tag="t6", name="ire")
                    iim = psA.tile([128, 512], FP, tag="t7", name="iim")
                    for b4 in range(2):
                        qt_re = p_bk.tile([64, 512], FP, tag="qt_re", name=f"qt_re{i}_{b4}")
                        qt_im = p_bk.tile([64, 512], FP, tag="qt_im", name=f"qt_im{i}_{b4}")
                        for srcf, dst in [(qf_re, qt_re), (qf_im, qt_im)]:
                            pt = psA.tile([64, 512], FP, tag="t3", name="ptq")
                            for k in range(4):
                                bkk = b4 * 4 + k
                                nc.tensor.transpose(
                                    pt[:, k * 128 : (k + 1) * 128],
                                    srcf[:, bkk * 64 : (bkk + 1) * 64], ct["ident"][:],
                                )
                            nc.scalar.copy(dst[:], pt[:])
                        for pi in range(4):
                            bkpi = b4 * 4 + pi
                            lre = qt_re[:, pi * 128 : (pi + 1) * 128]
                            lim = qt_im[:, pi * 128 : (pi + 1) * 128]
                            osl = slice(bkpi * 64, (bkpi + 1) * 64)
                            nc.tensor.matmul(ire[:, osl], lre, ct["e64_re"][:], start=True, stop=False)
                            nc.tensor.matmul(ire[:, osl], lim, ct["e64_im_neg"][:], start=False, stop=True)
                            nc.tensor.matmul(iim[:, osl], lre, ct["e64_im"][:], start=True, stop=False)
                            nc.tensor.matmul(iim[:, osl], lim, ct["e64_re"][:], start=False, stop=True)
                    in_re = p_bk.tile([128, 512], FP, tag="in_re", name=f"in_re{i}")
                    in_im = p_bk.tile([128, 512], FP, tag="in_im", name=f"in_im{i}")
                    t_a = p_bk.tile([128, 512], FP, tag="t_a", name="t_a")
                    ire3 = ire[:].rearrange("p (r t) -> p r t", t=P2)
                    iim3 = iim[:].rearrange("p (r t) -> p r t", t=P2)
                    ta3 = t_a[:].rearrange("p (r t) -> p r t", t=P2)
                    inre3 = in_re[:].rearrange("p (r t) -> p r t", t=P2)
                    inim3 = in_im[:].rearrange("p (r t) -> p r t", t=P2)
                    nc.vector.tensor_mul(ta3, ire3, twir_b)
                    nc.vector.tensor_mul(inre3, iim3, twii_b)
                    nc.vector.tensor_sub(inre3, ta3, inre3)
                    nc.vector.tensor_mul(ta3, ire3, twii_b)
                    nc.vector.tensor_mul(inim3, iim3, twir_b)
                    nc.vector.tensor_add(inim3, inim3, ta3)
                    # inverse outer -> tt_i [NT1, NBK*P2]
                    ot = psA.tile([NT1, 512], FP, tag="t4", name="ot")
                    nc.tensor.matmul(ot[:], ct["e128_re"][:], in_re[:], start=True, stop=False)
                    nc.tensor.matmul(ot[:], ct["e128_im_neg"][:], in_im[:], start=False, stop=True)
                    tt_i = p_bk.tile([NT1, 512], FP, tag="tt", name=f"tt{i}")
                    nc.scalar.copy(tt_i[:], ot[:])
                    # per-pair table: [NBK, NTAB] via 64 small transposes
                    tt3 = tt_i[:].rearrange("a (bk t) -> a bk t", bk=NBK)
                    ptab = psA.tile([NBK, NTAB], FP, tag="t5", name="ptab")
                    for t2v in range(P2):
                        nc.tensor.transpose(
                            ptab[:, t2v * NT1 : (t2v + 1) * NT1],
                            tt3[:, :, t2v : t2v + 1],
                            ct["ident"][0:NT1, 0:NT1],
                        )
                    tabs_i = p_bk.tile([NBK, NTAB], FP, tag="tabs", name=f"tabs{i}")
                    nc.scalar.copy(tabs_i[:], ptab[:])
                    # replicate each bk row across its 16 partitions
                    for halfc in range(2):
                        sl = slice(halfc * 512, (halfc + 1) * 512)
                        prep = psA.tile([128, 512], FP, tag="t6", name="prep")
                        nc.tensor.matmul(prep[:], ct["repl16"][:], tabs_i[:, sl])
                        nc.scalar.copy(gdata[i][:, sl], prep[:])
                    if i == 0:
                        # pair 0 gathers straight into the accumulator
                        for gh in range(2):
                            hsl = slice(gh * NQH, (gh + 1) * NQH)
                            if gh == 0:
                                nc.gpsimd.load_library(library_config.ap_gather)
                            nc.gpsimd.ap_gather(
                                acc[:, hsl], gdata[i][:],
                                gidx[i][:, gh * 512 : (gh + 1) * 512],
                                channels=128, num_elems=NTAB, d=1, num_idxs=NQH,
                            )
                    else:
                        for gh in range(2):
                            hsl = slice(gh * NQH, (gh + 1) * NQH)
                            gq = p_gb.tile([128, NQH], FP, tag="gout", name="gout")
                            nc.gpsimd.ap_gather(
                                gq[:], gdata[i][:],
                                gidx[i][:, gh * 512 : (gh + 1) * 512],
                                channels=128, num_elems=NTAB, d=1, num_idxs=NQH,
                            )
                            nc.vector.tensor_add(acc[:, hsl], acc[:, hsl], gq[:])

            p_bk.close()
            p_n.close()
            p_uv.close()
            p_yt.close()
            psA.close()
            p_z.close()
            p_a.close()
            p_ytw.close()
            p_gb.close()

            # ---------------- normalize + out ----------------
            with ExitStack() as gph:
                op = gph.enter_context(tc.tile_pool(name="gout2", bufs=1))
                psn = gph.enter_context(tc.tile_pool(name="psn", bufs=1, space="PSUM"))
                m128 = op.tile([128, 1], FP, tag="m128", name="m128")
                nc.vector.tensor_reduce(
                    m128[:], acc[:], axis=mybir.AxisListType.X, op=ALU.max
                )
                mt = psn.tile([1, 128], FP, tag="mt", name="mt")
                nc.tensor.transpose(mt[:], m128[:], ct["ident"][:])
                mg = op.tile([1, NBK], FP, tag="mg", name="mg")
                nc.vector.tensor_reduce(
                    mg[:],
                    mt[:].rearrange("a (g r) -> a g r", r=16),
                    axis=mybir.AxisListType.X, op=ALU.max,
                )
                nc.vector.reciprocal(mg[:], mg[:])
                mgt = psn.tile([NBK, 1], FP, tag="mgt", name="mgt")
                nc.tensor.transpose(mgt[:], mg[:], ct["ident"][0:1, 0:1])
                mgs = op.tile([NBK, 1], FP, tag="mgs", name="mgs")
                nc.scalar.copy(mgs[:], mgt[:])
                scp = psn.tile([128, 1], FP, tag="scp", name="scp")
                nc.tensor.matmul(scp[:], ct["repl16"][:], mgs[:])
                sc = op.tile([128, 1], FP, tag="sc", name="sc")
                nc.scalar.copy(sc[:], scp[:])
                for ch in range(NQ // 4096):
                    sl = slice(ch * 4096, (ch + 1) * 4096)
                    grids = op.tile([128, 4096], FP, tag="grids", name="grids")
                    nc.scalar.activation(
                        grids[:], acc[:, sl], ACTF.Copy, scale=sc[:, 0:1]
                    )
                    gv = grids[:].rearrange("(g r) q -> g r q", r=16)[:, 0, :]
                    nc.sync.dma_start(out_d[:, sl], gv)
            p_q.close()
            p_go.close()

    nc.compile()
    return nc


_NC_CACHE = {}


def kernel(signal, mic_coordinates, room_dims):
    signal = np.ascontiguousarray(np.asarray(signal, dtype=np.float32))
    mic_coordinates = np.ascontiguousarray(np.asarray(mic_coordinates, dtype=np.float32))
    room_dims = np.ascontiguousarray(np.asarray(room_dims, dtype=np.float32))
    if "nc" not in _NC_CACHE:
        _NC_CACHE["nc"] = build_kernel()
        _NC_CACHE["consts"] = make_consts()
    nc = _NC_CACHE["nc"]
    consts = _NC_CACHE["consts"]
    in_maps = []
    for core in range(8):
        m = shard_core(signal, mic_coordinates, room_dims, core)
        m.update(consts)
        in_maps.append(m)
    res = run_bass_kernel_spmd(nc, in_maps, core_ids=list(range(8)), trace=False)
    outs = [res.results[c]["out"].reshape(NB * K, NQ) for c in range(8)]
    return np.concatenate(outs, axis=0).reshape(B, K, NQ).astype(np.float32)



# revision 7
# speedup vs baseline: 1.0466x; 1.0017x over previous
"""GCC-PHAT spatial likelihood grid kernel for Trainium2 (8 NeuronCores).

Self-contained: kernel(**inputs) -> np.ndarray. Shards the batch over the 8
cores (pure data parallel), runs a Bass/Tile kernel per core, gathers.
"""

import json

import numpy as np
from contextlib import ExitStack

import concourse.bass as bass
import concourse.bacc as bacc
import concourse.mybir as mybir
from concourse import tile, library_config
from concourse.bass_utils import run_bass_kernel_spmd
from concourse.tile import TileContext
from bass_rust import ScopedClock

# ---------------------------------------------------------------------------
# Workaround 1: this walrus build allows at most one semaphore wait per
# instruction. Post-process the BIR JSON: excess waits move onto NoOps
# inserted just before the offending instruction (same engine, so ordering
# is preserved).
_uid = [0]


def _fix_module(m):
    for f in m.get("functions", []):
        for bb in f.get("blocks", []):
            insts = bb.get("instructions")
            if not insts:
                continue
            out = []
            changed = False
            for ins in insts:
                si = ins.get("sync_info")
                ow = (si or {}).get("on_wait") or []
                if len(ow) > 1:
                    changed = True
                    for w in ow[1:]:
                        _uid[0] += 1
                        out.append({
                            "engine": ins["engine"], "ins": [], "outs": [],
                            "name": f"WFix-{_uid[0]}", "opcode": "NoOp",
                            "sync_info": {"on_update": [], "on_wait": [w]},
                        })
                    si["on_wait"] = ow[:1]
                out.append(ins)
            if changed:
                bb["instructions"] = out
    return m


_orig_to_json_bytes = bass.Bass.to_json_bytes


def _to_json_bytes(self):
    return json.dumps(_fix_module(json.loads(_orig_to_json_bytes(self)))).encode()


bass.Bass.to_json_bytes = _to_json_bytes

# Workaround 2: the TileContext tail Drain gets zero wait slots here; move
# the end-of-kernel waits onto SP NoOps emitted before the drain.


def _drain_and_barrier(self, tick_clock, wait_clock):
    nc = self.nc
    first_nop = nc.sync.nop()
    wait_clock.add_sem_waits(first_nop.ins, ScopedClock({None: tick_clock.global_clock}))
    si = first_nop.ins.sync_info
    if si is not None and len(si.on_wait) > 1:
        waits = list(si.on_wait)
        first_nop.ins.sync_info = mybir.SyncInfo(
            on_wait=waits[:1], on_update=list(si.on_update)
        )
        for w in waits[1:]:
            nop = nc.sync.nop()
            nop.ins.sync_info = mybir.SyncInfo(on_wait=[w], on_update=[])
    nc.sync.drain()
    nc.all_engine_barrier()
    popped = nc._tile_sem_poison_stack.pop()
    assert popped is self._sem_poison
    nc.clear_and_free_semaphores(list(self.sems.allocated().values()))
    nc.all_engine_barrier()


TileContext._drain_and_barrier = _drain_and_barrier

# ---------------------------------------------------------------------------
FP = mybir.dt.float32
I32 = mybir.dt.int32
I16 = mybir.dt.int16
ALU = mybir.AluOpType
ACTF = mybir.ActivationFunctionType

SR = 16000.0
SPEED = 343.0
G = 128
B, K, M, N = 16, 4, 4, 8192
P1, P2 = 128, 64  # N = P1*P2 ; n = p*64 + j ; bin k = k2*128 + k1
PAIRS = [(0, 1), (0, 2), (0, 3), (1, 2), (1, 3), (2, 3)]
NPA = 6
NB = 2
NBK = NB * K  # 8
NS = NBK * M  # 32
NPAIR = NBK * NPA  # 48
T1LO, T1HI = 56, 72
NT1 = T1HI - T1LO  # 16
NTAB = NT1 * P2  # 1024
NQ = G * G  # 16384
G2HI = G // 16  # 8


def make_consts():
    c = {}
    k1 = np.arange(P1)
    j = np.arange(P2)
    k2 = np.arange(P2)
    t2 = np.arange(P2)
    p = np.arange(P1)

    ang = 2 * np.pi * np.outer(p, k1) / P1
    c["a_re"] = np.cos(ang).astype(np.float32)  # [p, k1]
    c["a_im"] = (-np.sin(ang)).astype(np.float32)

    angt = 2 * np.pi * np.outer(k1, j) / N  # fwd twiddle e^{-i...}
    c["twf_re"] = np.tile(np.cos(angt).astype(np.float32), (1, NS))  # [128, 2048]
    c["twf_im"] = np.tile((-np.sin(angt)).astype(np.float32), (1, NS))

    angb = 2 * np.pi * np.outer(j, k2) / P2  # W64 = e^{-i...}
    c["b_re"] = np.cos(angb).astype(np.float32)  # [j, k2]
    c["b_im"] = (-np.sin(angb)).astype(np.float32)
    c["b_im_neg"] = -c["b_im"]

    kk = k1[:, None] + P1 * k2[None, :]  # [k1, k2] bin index
    o = np.arange(-5, 5)
    D = np.exp(2j * np.pi * kk[..., None] * o / N).sum(-1)
    DW = D * ((-1.0) ** kk) / N
    c["dw_re"] = np.tile(np.ascontiguousarray(DW.real).astype(np.float32), (1, NS))
    c["dw_im"] = np.tile(np.ascontiguousarray(DW.imag).astype(np.float32), (1, NS))

    angi = 2 * np.pi * np.outer(k2, t2) / P2  # E64 = e^{+i...}
    c["e64_re"] = np.cos(angi).astype(np.float32)  # [k2, t2]
    c["e64_im"] = np.sin(angi).astype(np.float32)
    c["e64_im_neg"] = -c["e64_im"]

    angti = 2 * np.pi * np.outer(k1, t2) / N  # inv twiddle e^{+i...}
    c["twi_re"] = np.tile(np.cos(angti).astype(np.float32), (1, NPAIR))  # [128, 3072]
    c["twi_im"] = np.tile(np.sin(angti).astype(np.float32), (1, NPAIR))

    t1 = np.arange(T1LO, T1HI)
    ango = 2 * np.pi * np.outer(k1, t1) / P1  # E128 = e^{+i...}
    c["e128_re"] = np.cos(ango).astype(np.float32)  # [k1, 16]
    c["e128_im_neg"] = (-np.sin(ango)).astype(np.float32)

    c["ident"] = np.eye(128, dtype=np.float32)

    # group-broadcast selector: lhsT [8, 128]; out row p <- in row p//16
    r16 = np.zeros((NBK, 128), np.float32)
    for g in range(NBK):
        r16[g, g * 16 : (g + 1) * 16] = 1.0
    c["repl16"] = r16

    # grid parameter fields (wrapped layout): partition = (bk, g2lo), free = (g1, g2hi)
    t_lin = np.linspace(0.0, 1.0, G).astype(np.float32)
    g2lo = (p % 16)[:, None, None]
    g1 = np.arange(G)[None, :, None]
    g2hi = np.arange(G2HI)[None, None, :]
    gxc = np.broadcast_to(t_lin[g1], (128, G, G2HI))
    gyc = t_lin[(g2hi * 16 + g2lo)] * np.ones((128, G, G2HI), np.float32)
    c["gxc"] = np.ascontiguousarray(gxc.reshape(128, NQ // 16), np.float32)
    c["gyc"] = np.ascontiguousarray(gyc.reshape(128, NQ // 16), np.float32)
    return c


CONST_SPECS = [
    ("a_re", [128, 128]), ("a_im", [128, 128]),
    ("twf_re", [128, 64 * NS]), ("twf_im", [128, 64 * NS]),
    ("b_re", [64, 64]), ("b_im", [64, 64]), ("b_im_neg", [64, 64]),
    ("dw_re", [128, 64 * NS]), ("dw_im", [128, 64 * NS]),
    ("e64_re", [64, 64]), ("e64_im", [64, 64]), ("e64_im_neg", [64, 64]),
    ("twi_re", [128, 64 * NPAIR]), ("twi_im", [128, 64 * NPAIR]),
    ("e128_re", [128, NT1]), ("e128_im_neg", [128, NT1]),
    ("ident", [128, 128]),
    ("repl16", [NBK, 128]),
    ("gxc", [128, NQ // 16]), ("gyc", [128, NQ // 16]),
]


def shard_core(signal, mic, room, core):
    b0 = core * NB
    sig = np.ascontiguousarray(signal[b0 : b0 + NB].transpose(2, 0, 1, 3).reshape(NS, N))
    micc = mic[b0 : b0 + NB]
    roomc = room[b0 : b0 + NB]
    pp = np.arange(128)
    bk = pp // 16
    b = bk // K
    k = bk % K
    aux = np.zeros((128, 12), np.float32)
    aux[:, 0] = roomc[b, 0]
    aux[:, 1] = roomc[b, 1]
    for m in range(M):
        aux[:, 2 + m] = micc[b, k, m, 0]
        aux[:, 6 + m] = micc[b, k, m, 1]
    return {"signal": sig, "aux": aux}


def build_kernel():
    nc = bacc.Bacc("TRN2", target_bir_lowering=False, debug=False)
    sig_d = nc.declare_dram_parameter("signal", [NS, N], FP, isOutput=False)
    aux_d = nc.declare_dram_parameter("aux", [128, 12], FP, isOutput=False)
    cd = {
        name: nc.declare_dram_parameter(name, shape, FP, isOutput=False)
        for name, shape in CONST_SPECS
    }
    out_d = nc.declare_dram_parameter("out", [NBK, NQ], FP, isOutput=True)

    with tile.TileContext(nc) as tc:
        with ExitStack() as top:
            # small constants that live for the whole kernel
            cpool = top.enter_context(tc.tile_pool(name="consts", bufs=1))
            SMALL = {"a_re", "a_im", "b_re", "b_im", "b_im_neg", "e64_re", "e64_im",
                     "e64_im_neg", "e128_re", "e128_im_neg", "ident", "repl16"}
            ct = {}
            for name, shape in CONST_SPECS:
                if name in SMALL:
                    t = cpool.tile(shape, FP, tag=name, name=name)
                    nc.sync.dma_start(t[:], cd[name][:, :])
                    ct[name] = t
            aux = cpool.tile([128, 12], FP, tag="aux", name="aux")
            nc.sync.dma_start(aux[:], aux_d[:, :])
            bias_round = cpool.tile([128, 1], FP, tag="bias_round", name="bias_round")
            nc.vector.memset(bias_round[:], 512.0)
            bias_eps = cpool.tile([128, 1], FP, tag="bias_eps", name="bias_eps")
            nc.vector.memset(bias_eps[:], 1e-18)

            # gather inputs persist until the gather phase
            gpool = top.enter_context(tc.tile_pool(name="gin", bufs=1))
            gdata = [gpool.tile([128, NTAB], FP, tag=f"gdata{i}", name=f"gdata{i}") for i in range(NPA)]
            gidx = [gpool.tile([128, NTAB], I16, tag=f"gidx{i}", name=f"gidx{i}") for i in range(NPA)]

            # ---------------- geometry / index pipeline ----------------
            with ExitStack() as geo:
                gp = geo.enter_context(tc.tile_pool(name="geo", bufs=1))
                NF = NQ // 16  # 1024
                gxc = gp.tile([128, NF], FP, tag="gxc", name="gxc")
                nc.sync.dma_start(gxc[:], cd["gxc"][:, :])
                gyc = gp.tile([128, NF], FP, tag="gyc", name="gyc")
                nc.sync.dma_start(gyc[:], cd["gyc"][:, :])
                gx = gp.tile([128, NF], FP, tag="gx", name="gx")
                gy = gp.tile([128, NF], FP, tag="gy", name="gy")
                nc.vector.tensor_scalar_mul(gx[:], gxc[:], aux[:, 0:1])
                nc.vector.tensor_scalar_mul(gy[:], gyc[:], aux[:, 1:2])
                dist = [gp.tile([128, NF], FP, tag=f"dist{m}", name=f"dist{m}") for m in range(M)]
                negm = gp.tile([128, 8], FP, tag="negm", name="negm")
                nc.vector.tensor_scalar_mul(negm[:], aux[:, 2:10], -1.0)
                for m in range(M):
                    d2 = gp.tile([128, NF], FP, tag="d2", name="d2")
                    dy2 = gp.tile([128, NF], FP, tag="dy2", name="dy2")
                    nc.scalar.activation(d2[:], gx[:], ACTF.Square, bias=negm[:, m : m + 1])
                    nc.scalar.activation(
                        dy2[:], gy[:], ACTF.Square, bias=negm[:, 4 + m : 5 + m]
                    )
                    nc.vector.tensor_add(d2[:], d2[:], dy2[:])
                    nc.scalar.activation(dist[m][:], d2[:], ACTF.Sqrt)
                for i, (mi, mj) in enumerate(PAIRS):
                    ds = gp.tile([128, NF], FP, tag="ds", name="ds")
                    dsi = gp.tile([128, NF], I32, tag="dsi", name="dsi")
                    d16 = gp.tile([128, NF], I32, tag="d16", name="d16")
                    dl = gp.tile([128, NF], I32, tag="dl", name="dl")
                    nc.vector.tensor_sub(ds[:], dist[mi][:], dist[mj][:])
                    # dlocal+0.5 = ds*(SR/343) + 512.5 ; then clamp, truncate
                    nc.scalar.activation(
                        ds[:], ds[:], ACTF.Identity,
                        bias=bias_round[:, 0:1], scale=float(np.float32(SR) / np.float32(SPEED)),
                    )
                    # no clamp: |dist_i - dist_j| < sqrt(72) m in a <=6m room,
                    # so the index stays within [116, 908] of the 1024 window
                    nc.vector.tensor_copy(dsi[:], ds[:])
                    # permuted index d' = ((d*16) & 1023) + ((d*16) >> 10)
                    nc.vector.tensor_scalar(d16[:], dsi[:], 16, None, op0=ALU.mult)
                    nc.vector.tensor_scalar(dl[:], d16[:], 1023, None, op0=ALU.bitwise_and)
                    nc.vector.tensor_scalar(
                        d16[:], d16[:], 10, None, op0=ALU.logical_shift_right
                    )
                    nc.vector.tensor_tensor(gidx[i][:], d16[:], dl[:], op=ALU.add)

            # ------------- forward FFT + spectra + inverse (half/pair split) ----
            # Signal order is now s = m*NBK + bk (mics major), so half h of the
            # signal range covers mics {2h, 2h+1}. Front tiles are split per
            # half and the post-PHAT stages per pair so that pair 0's table
            # (and its gather) is ready long before the full front finishes.
            class MPool:
                def __init__(self, name, space="SBUF", bufs=1, side=None):
                    self.cm = tc.tile_pool(name=name, bufs=bufs, space=space, side=side)
                    self.pool = self.cm.__enter__()
                def tile(self, *a, **kw):
                    return self.pool.tile(*a, **kw)
                def close(self):
                    self.cm.__exit__(None, None, None)

            NSH = NS // 2  # 16 signals per half
            HC = NSH * P2  # 1024 cols per half

            # persistent-front pools (tiles span both halves; h-major order
            # below puts mic-{0,1} FFT + pair 0's whole chain + gather 0
            # ahead of the mic-{2,3} FFT in every engine queue)
            p_go = MPool("p_go", side="left", bufs=1)
            acc = p_go.tile([128, NQ], FP, tag="acc", name="acc")
            m128a = p_go.tile([128, 1], FP, tag="m128a", name="m128a")
            p_q = MPool("p_q", side="left", bufs=2)
            p_gb = MPool("p_gb", side="left", bufs=1)
            p_ytw = MPool("p_ytw", side="left")
            p_a = MPool("p_a", side="left")
            psA = MPool("psA", space="PSUM", bufs=1)
            twf_re = p_a.tile([128, P2], FP, name="twf_re")
            nc.sync.dma_start(twf_re[:], cd["twf_re"][:, 0:P2])
            twf_im = p_a.tile([128, P2], FP, name="twf_im")
            nc.sync.dma_start(twf_im[:], cd["twf_im"][:, 0:P2])
            twfr_b = twf_re[:].rearrange("p (o j) -> p o j", o=1).broadcast_to((128, 8, P2))
            twfi_b = twf_im[:].rearrange("p (o j) -> p o j", o=1).broadcast_to((128, 8, P2))
            p_yt = MPool("p_yt", side="right")

            p_z = MPool("p_z", side="left")
            p_uv = MPool("p_uv", side="right")
            u_mre, u_mim, v_mre, v_mim = {}, {}, {}, {}
            p_n = MPool("p_n", side="right", bufs=1)
            dw_res = p_n.tile([128, P2], FP, tag="dwr", name="dw_res")
            nc.sync.dma_start(dw_res[:], cd["dw_re"][:, 0:P2])
            dw_ims = p_n.tile([128, P2], FP, tag="dwi", name="dw_ims")
            nc.sync.dma_start(dw_ims[:], cd["dw_im"][:, 0:P2])
            dw_re3 = dw_res[:].rearrange("p (o j) -> p o j", o=1).broadcast_to((128, NSH, P2))
            dw_im3 = dw_ims[:].rearrange("p (o j) -> p o j", o=1).broadcast_to((128, NSH, P2))
            p_bk = MPool("p_bk", side="right", bufs=1)
            twi_res = p_bk.tile([128, P2], FP, tag="twir", name="twi_res")
            nc.sync.dma_start(twi_res[:], cd["twi_re"][:, 0:P2])
            twi_ims = p_bk.tile([128, P2], FP, tag="twii", name="twi_ims")
            nc.sync.dma_start(twi_ims[:], cd["twi_im"][:, 0:P2])
            twir_b = twi_res[:].rearrange("p (o t) -> p o t", o=1).broadcast_to((128, 8, P2))
            twii_b = twi_ims[:].rearrange("p (o t) -> p o t", o=1).broadcast_to((128, 8, P2))
            NQH = NQ // 2
            NQQ = NQ // 4

            def mview(tiles, m):
                return tiles[m][:].rearrange("p (bk k) -> p bk k", bk=NBK)

            for h in range(2):

                # step A + twiddle for this half
                x2h = p_a.tile([128, NSH, P2], FP, tag="x2", name=f"x2{h}")
                nc.sync.dma_start(
                    x2h[:],
                    sig_d[h * NSH : (h + 1) * NSH, :].rearrange("s (p j) -> p s j", p=128),
                )
                x2f = x2h[:].rearrange("p s j -> p (s j)")
                z_lc_re, z_lc_im = {}, {}
                for lc in range(2):
                    sl = slice(lc * 512, (lc + 1) * 512)
                    yre = psA.tile([128, 512], FP, tag="t1", name="yre")
                    yim = psA.tile([128, 512], FP, tag="t2", name="yim")
                    nc.tensor.matmul(yre[:], ct["a_re"][:], x2f[:, sl])
                    nc.tensor.matmul(yim[:], ct["a_im"][:], x2f[:, sl])
                    tmp = p_a.tile([128, 512], FP, tag="twtmp", name="twtmp")
                    ytwr = p_ytw.tile([128, 512], FP, tag="ytw_re", name=f"ytw_re{h}{lc}")
                    ytwi = p_ytw.tile([128, 512], FP, tag="ytw_im", name=f"ytw_im{h}{lc}")
                    yre3 = yre[:].rearrange("p (s j) -> p s j", j=P2)
                    yim3 = yim[:].rearrange("p (s j) -> p s j", j=P2)
                    tmp3 = tmp[:].rearrange("p (s j) -> p s j", j=P2)
                    ytw_re3 = ytwr[:].rearrange("p (s j) -> p s j", j=P2)
                    ytw_im3 = ytwi[:].rearrange("p (s j) -> p s j", j=P2)
                    nc.vector.tensor_mul(tmp3, yre3, twfr_b)
                    nc.vector.tensor_mul(ytw_re3, yim3, twfi_b)
                    nc.vector.tensor_sub(ytw_re3, tmp3, ytw_re3)
                    nc.vector.tensor_mul(tmp3, yre3, twfi_b)
                    nc.vector.tensor_mul(ytw_im3, yim3, twfr_b)
                    nc.vector.tensor_add(ytw_im3, tmp3, ytw_im3)
                    # transpose + step B, one 4-signal chunk at a time
                    zre = psA.tile([128, 512], FP, tag="t1", name="zre")
                    zim = psA.tile([128, 512], FP, tag="t2", name="zim")
                    for c4 in range(2):
                        yTr = p_yt.tile([64, 512], FP, tag="ytTc_re", name=f"yTr{h}{lc}{c4}")
                        yTi = p_yt.tile([64, 512], FP, tag="ytTc_im", name=f"yTi{h}{lc}{c4}")
                        for srcv, dst in [(ytwr, yTr), (ytwi, yTi)]:
                            pt = psA.tile([64, 512], FP, tag="t3", name="ptr")
                            for k in range(4):
                                ls = c4 * 4 + k
                                nc.tensor.transpose(
                                    pt[:, k * 128 : (k + 1) * 128],
                                    srcv[:, ls * 64 : (ls + 1) * 64], ct["ident"][:],
                                )
                            nc.scalar.copy(dst[:], pt[:])
                        for col in range(4):
                            si = c4 * 4 + col
                            lre = yTr[:, col * 128 : (col + 1) * 128]
                            lim = yTi[:, col * 128 : (col + 1) * 128]
                            osl = slice(si * 64, (si + 1) * 64)
                            nc.tensor.matmul(zre[:, osl], lre, ct["b_re"][:], start=True, stop=False)
                            nc.tensor.matmul(zre[:, osl], lim, ct["b_im_neg"][:], start=False, stop=True)
                            nc.tensor.matmul(zim[:, osl], lre, ct["b_im"][:], start=True, stop=False)
                            nc.tensor.matmul(zim[:, osl], lim, ct["b_re"][:], start=False, stop=True)
                    z_lc_re[lc] = p_z.tile([128, 512], FP, tag="z_re", name=f"z_re{h}{lc}")
                    z_lc_im[lc] = p_z.tile([128, 512], FP, tag="z_im", name=f"z_im{h}{lc}")
                    nc.scalar.copy(z_lc_re[lc][:], zre[:])
                    nc.scalar.copy(z_lc_im[lc][:], zim[:])
                # PHAT + Dirichlet per mic (chunk ph <-> mic 2h+ph); V_3 unused
                for ph in range(2):
                    m = 2 * h + ph
                    nrm = p_n.tile([128, 512], FP, tag="nrm", name="nrm")
                    tmp2 = p_n.tile([128, 512], FP, tag="tmp2", name="tmp2")
                    nc.scalar.activation(nrm[:], z_lc_re[ph][:], ACTF.Square)
                    nc.scalar.activation(tmp2[:], z_lc_im[ph][:], ACTF.Square)
                    nc.vector.tensor_add(nrm[:], nrm[:], tmp2[:])
                    nc.scalar.activation(nrm[:], nrm[:], ACTF.Sqrt, bias=bias_eps[:, 0:1])
                    nc.vector.reciprocal(nrm[:], nrm[:])
                    u_mre[m] = p_uv.tile([128, 512], FP, tag=f"um_re{m % 2}", name=f"u_re_m{m}")
                    u_mim[m] = p_uv.tile([128, 512], FP, tag=f"um_im{m % 2}", name=f"u_im_m{m}")
                    nc.vector.tensor_mul(u_mre[m][:], z_lc_re[ph][:], nrm[:])
                    nc.vector.tensor_mul(u_mim[m][:], z_lc_im[ph][:], nrm[:])
                    if m < 3:
                        v_mre[m] = p_uv.tile([128, 512], FP, tag=f"vm_re{m}", name=f"v_re_m{m}")
                        v_mim[m] = p_uv.tile([128, 512], FP, tag=f"vm_im{m}", name=f"v_im_m{m}")
                        ur3 = u_mre[m][:].rearrange("p (s j) -> p s j", j=P2)
                        ui3 = u_mim[m][:].rearrange("p (s j) -> p s j", j=P2)
                        vr3 = v_mre[m][:].rearrange("p (s j) -> p s j", j=P2)
                        vi3 = v_mim[m][:].rearrange("p (s j) -> p s j", j=P2)
                        tm3 = tmp2[:].rearrange("p (s j) -> p s j", j=P2)
                        dw_re8 = dw_res[:].rearrange("p (o j) -> p o j", o=1).broadcast_to((128, 8, P2))
                        dw_im8 = dw_ims[:].rearrange("p (o j) -> p o j", o=1).broadcast_to((128, 8, P2))
                        nc.vector.tensor_mul(vr3, ur3, dw_re8)
                        nc.vector.tensor_mul(tm3, ui3, dw_im8)
                        nc.vector.tensor_sub(vr3, vr3, tm3)
                        nc.vector.tensor_mul(vi3, ur3, dw_im8)
                        nc.vector.tensor_mul(tm3, ui3, dw_re8)
                        nc.vector.tensor_add(vi3, vi3, tm3)

                # pairs available after this half, with full back chain+gather
                for i in ([0] if h == 0 else [1, 2, 3, 4, 5]):
                    mi, mj = PAIRS[i]
                    vi_re, vi_im = mview(v_mre, mi), mview(v_mim, mi)
                    uj_re, uj_im = mview(u_mre, mj), mview(u_mim, mj)
                    q_re_i = p_q.tile([128, NBK, P2], FP, tag="q_re", name=f"q_re{i}")
                    q_im_i = p_q.tile([128, NBK, P2], FP, tag="q_im", name=f"q_im{i}")
                    tq = p_q.tile([128, NBK, P2], FP, tag="qtmp", name="qtmp")
                    nc.vector.tensor_mul(q_re_i[:], vi_re, uj_re)
                    nc.vector.tensor_mul(tq[:], vi_im, uj_im)
                    nc.vector.tensor_add(q_re_i[:], q_re_i[:], tq[:])
                    nc.vector.tensor_mul(q_im_i[:], vi_im, uj_re)
                    nc.vector.tensor_mul(tq[:], vi_re, uj_im)
                    nc.vector.tensor_sub(q_im_i[:], q_im_i[:], tq[:])
                    # transpose Q_i -> Qt + inverse inner, per bk-half
                    qf_re = q_re_i[:].rearrange("p bk k -> p (bk k)")
                    qf_im = q_im_i[:].rearrange("p bk k -> p (bk k)")
                    ire = psA.tile([128, 512], FP, tag="t6", name="ire")
                    iim = psA.tile([128, 512], FP, tag="t7", name="iim")
                    for b4 in range(2):
                        qt_re = p_bk.tile([64, 512], FP, tag="qt_re", name=f"qt_re{i}_{b4}")
                        qt_im = p_bk.tile([64, 512], FP, tag="qt_im", name=f"qt_im{i}_{b4}")
                        for srcf, dst in [(qf_re, qt_re), (qf_im, qt_im)]:
                            pt = psA.tile([64, 512], FP, tag="t3", name="ptq")
                            for k in range(4):
                                bkk = b4 * 4 + k
                                nc.tensor.transpose(
                                    pt[:, k * 128 : (k + 1) * 128],
                                    srcf[:, bkk * 64 : (bkk + 1) * 64], ct["ident"][:],
                                )
                            nc.scalar.copy(dst[:], pt[:])
                        for pi in range(4):
                            bkpi = b4 * 4 + pi
                            lre = qt_re[:, pi * 128 : (pi + 1) * 128]
                            lim = qt_im[:, pi * 128 : (pi + 1) * 128]
                            osl = slice(bkpi * 64, (bkpi + 1) * 64)
                            nc.tensor.matmul(ire[:, osl], lre, ct["e64_re"][:], start=True, stop=False)
                            nc.tensor.matmul(ire[:, osl], lim, ct["e64_im_neg"][:], start=False, stop=True)
                            nc.tensor.matmul(iim[:, osl], lre, ct["e64_im"][:], start=True, stop=False)
                            nc.tensor.matmul(iim[:, osl], lim, ct["e64_re"][:], start=False, stop=True)
                    in_re = p_bk.tile([128, 512], FP, tag="in_re", name=f"in_re{i}")
                    in_im = p_bk.tile([128, 512], FP, tag="in_im", name=f"in_im{i}")
                    t_a = p_bk.tile([128, 512], FP, tag="t_a", name="t_a")
                    ire3 = ire[:].rearrange("p (r t) -> p r t", t=P2)
                    iim3 = iim[:].rearrange("p (r t) -> p r t", t=P2)
                    ta3 = t_a[:].rearrange("p (r t) -> p r t", t=P2)
                    inre3 = in_re[:].rearrange("p (r t) -> p r t", t=P2)
                    inim3 = in_im[:].rearrange("p (r t) -> p r t", t=P2)
                    nc.vector.tensor_mul(ta3, ire3, twir_b)
                    nc.vector.tensor_mul(inre3, iim3, twii_b)
                    nc.vector.tensor_sub(inre3, ta3, inre3)
                    nc.vector.tensor_mul(ta3, ire3, twii_b)
                    nc.vector.tensor_mul(inim3, iim3, twir_b)
                    nc.vector.tensor_add(inim3, inim3, ta3)
                    # inverse outer -> tt_i [NT1, NBK*P2]
                    ot = psA.tile([NT1, 512], FP, tag="t4", name="ot")
                    nc.tensor.matmul(ot[:], ct["e128_re"][:], in_re[:], start=True, stop=False)
                    nc.tensor.matmul(ot[:], ct["e128_im_neg"][:], in_im[:], start=False, stop=True)
                    tt_i = p_bk.tile([NT1, 512], FP, tag="tt", name=f"tt{i}")
                    nc.scalar.copy(tt_i[:], ot[:])
                    # per-pair table: [NBK, NTAB] via 64 small transposes
                    tt3 = tt_i[:].rearrange("a (bk t) -> a bk t", bk=NBK)
                    ptab = psA.tile([NBK, NTAB], FP, tag="t5", name="ptab")
                    for t2v in range(P2):
                        nc.tensor.transpose(
                            ptab[:, t2v * NT1 : (t2v + 1) * NT1],
                            tt3[:, :, t2v : t2v + 1],
                            ct["ident"][0:NT1, 0:NT1],
                        )
                    tabs_i = p_bk.tile([NBK, NTAB], FP, tag="tabs", name=f"tabs{i}")
                    nc.scalar.copy(tabs_i[:], ptab[:])
                    # replicate each bk row across its 16 partitions
                    for halfc in range(2):
                        sl = slice(halfc * 512, (halfc + 1) * 512)
                        prep = psA.tile([128, 512], FP, tag="t6", name="prep")
                        nc.tensor.matmul(prep[:], ct["repl16"][:], tabs_i[:, sl])
                        nc.scalar.copy(gdata[i][:, sl], prep[:])
                    if i == 0:
                        # pair 0 gathers straight into the accumulator
                        for gh in range(2):
                            hsl = slice(gh * NQH, (gh + 1) * NQH)
                            if gh == 0:
                                nc.gpsimd.load_library(library_config.ap_gather)
                            nc.gpsimd.ap_gather(
                                acc[:, hsl], gdata[i][:],
                                gidx[i][:, gh * 512 : (gh + 1) * 512],
                                channels=128, num_elems=NTAB, d=1, num_idxs=NQH,
                            )
                    else:
                        for gh in range(2):
                            hsl = slice(gh * NQH, (gh + 1) * NQH)
                            gq = p_gb.tile([128, NQH], FP, tag="gout", name="gout")
                            nc.gpsimd.ap_gather(
                                gq[:], gdata[i][:],
                                gidx[i][:, gh * 512 : (gh + 1) * 512],
                                channels=128, num_elems=NTAB, d=1, num_idxs=NQH,
                            )
                            nc.vector.tensor_add(acc[:, hsl], acc[:, hsl], gq[:])
                            if i == 5 and gh == 0:
                                nc.vector.tensor_reduce(
                                    m128a[:], acc[:, 0:NQH],
                                    axis=mybir.AxisListType.X, op=ALU.max,
                                )

            p_bk.close()
            p_n.close()
            p_uv.close()
            p_yt.close()
            psA.close()
            p_z.close()
            p_a.close()
            p_ytw.close()
            p_gb.close()

            # ---------------- normalize + out ----------------
            with ExitStack() as gph:
                op = gph.enter_context(tc.tile_pool(name="gout2", bufs=1))
                psn = gph.enter_context(tc.tile_pool(name="psn", bufs=1, space="PSUM"))
                m128 = op.tile([128, 1], FP, tag="m128", name="m128")
                nc.vector.tensor_reduce(
                    m128[:], acc[:, NQH:], axis=mybir.AxisListType.X, op=ALU.max
                )
                nc.vector.tensor_tensor(m128[:], m128[:], m128a[:], op=ALU.max)
                mt = psn.tile([1, 128], FP, tag="mt", name="mt")
                nc.tensor.transpose(mt[:], m128[:], ct["ident"][:])
                mg = op.tile([1, NBK], FP, tag="mg", name="mg")
                nc.vector.tensor_reduce(
                    mg[:],
                    mt[:].rearrange("a (g r) -> a g r", r=16),
                    axis=mybir.AxisListType.X, op=ALU.max,
                )
                nc.vector.reciprocal(mg[:], mg[:])
                mgt = psn.tile([NBK, 1], FP, tag="mgt", name="mgt")
                nc.tensor.transpose(mgt[:], mg[:], ct["ident"][0:1, 0:1])
                mgs = op.tile([NBK, 1], FP, tag="mgs", name="mgs")
                nc.scalar.copy(mgs[:], mgt[:])
                scp = psn.tile([128, 1], FP, tag="scp", name="scp")
                nc.tensor.matmul(scp[:], ct["repl16"][:], mgs[:])
                sc = op.tile([128, 1], FP, tag="sc", name="sc")
                nc.scalar.copy(sc[:], scp[:])
                for ch in range(NQ // 4096):
                    sl = slice(ch * 4096, (ch + 1) * 4096)
                    if ch < 2:
                        grids = op.tile([128, 4096], FP, tag="grids_s", name="grids_s")
                        nc.scalar.activation(
                            grids[:], acc[:, sl], ACTF.Copy, scale=sc[:, 0:1]
                        )
                    else:
                        grids = op.tile([128, 4096], FP, tag="grids_v", name="grids_v")
                        nc.vector.tensor_scalar_mul(grids[:], acc[:, sl], sc[:, 0:1])
                    gv = grids[:].rearrange("(g r) q -> g r q", r=16)[:, 0, :]
                    nc.sync.dma_start(out_d[:, sl], gv)
            p_q.close()
            p_go.close()

    nc.compile()
    return nc


_NC_CACHE = {}


def kernel(signal, mic_coordinates, room_dims):
    signal = np.ascontiguousarray(np.asarray(signal, dtype=np.float32))
    mic_coordinates = np.ascontiguousarray(np.asarray(mic_coordinates, dtype=np.float32))
    room_dims = np.ascontiguousarray(np.asarray(room_dims, dtype=np.float32))
    if "nc" not in _NC_CACHE:
        _NC_CACHE["nc"] = build_kernel()
        _NC_CACHE["consts"] = make_consts()
    nc = _NC_CACHE["nc"]
    consts = _NC_CACHE["consts"]
    in_maps = []
    for core in range(8):
        m = shard_core(signal, mic_coordinates, room_dims, core)
        m.update(consts)
        in_maps.append(m)
    res = run_bass_kernel_spmd(nc, in_maps, core_ids=list(range(8)), trace=False)
    outs = [res.results[c]["out"].reshape(NB * K, NQ) for c in range(8)]
    return np.concatenate(outs, axis=0).reshape(B, K, NQ).astype(np.float32)

